# revision 15
# baseline (speedup 1.0000x reference)
"""ContextualConv2d Trainium2 kernel.

out = conv2d(x, weight, pad=1) + (c @ c_weight.T)[:, :, None, None] + bias[None, :, None, None]

Full shapes: x (32,128,64,64) f32, c (32,64), weight (256,128,3,3),
c_weight (256,64), bias (256,) -> out (32,256,64,64).

Strategy: data-parallel over batch across 8 NeuronCores (4 images each).
Per core the conv is an implicit GEMM: each image lives in SBUF with
stride-65 rows (a host-baked zero guard column after each 64-pixel row,
plus two zero rows for the H halo), so the +-1-column filter taps read
straight through zero guards and every tap is a uniform N=512 matmul
with inner-contiguous rhs. For each 128-wide C_out tile and each
512-column output block (8 image rows x 64 cols), 9 matmuls (one per
filter tap) accumulate into a PSUM bank.

Conv operands are bf16 (~3e-3 rel err, well under the 2e-2 gate): bf16
moving streams at the full 1 elem/cycle PE rate with a ~216ns warm
back-to-back gap at N=512, vs ~237ns measured for fp32r (fp32-class
LDWEIGHTS is slower and only partially hidden), and input DMA bytes
halve. Weights are stored co-tile-major (t, tap, 128) and DMAed in two
halves so the first conv matmul only waits on the first 0.3MB; image 0
leads the sync ring in 3 interior pieces while the weights ride the
scalar ring in parallel. The context bias (c @ c_weight.T + bias) comes
from one small on-device fp32r matmul per C_out tile (a ones-row on the
rhs folds in the channel bias) and is fused into the PSUM->SBUF
epilogue on ACT (co-tile 0) / DVE (co-tile 1).

Schedule: 6 bf16 warmup matmuls keep the PE busy (HAM un-throttle)
while the first inputs stream; images 1-3 are prefetched one compute
pass ahead on the scalar ring; output planes are stored in 4 x 512KB
contiguous pieces, except the last plane which goes in 8 x 256KB pieces
with the final 256KB split across both rings so the kernel tail only
carries ~128KB of store. Baseline (fp32r, serial weight wait): ~159us.
"""

import sys
import time
import types

import numpy as np

import concourse.tile as tile
from concourse import bacc, bass_utils, mybir


def _ensure_axon_hooks_shim():
    """concourse imports antenv.axon_hooks when BASS_TRACE is set; the agent
    image's antenv lacks it. Provide a null shim so tracing degrades to a
    warning instead of an ImportError."""
    try:
        import antenv

        if not hasattr(antenv, "axon_hooks"):
            try:
                from antenv import axon_hooks  # noqa: F401
            except ImportError:
                mod = types.ModuleType("antenv.axon_hooks")
                _state = {"hook": None}
                mod.set_axon_ntff_profile_hook = lambda h: _state.__setitem__(
                    "hook", h
                )
                mod.get_axon_ntff_profile_hook = lambda: _state["hook"]
                sys.modules["antenv.axon_hooks"] = mod
                antenv.axon_hooks = mod
    except Exception:
        pass


_ensure_axon_hooks_shim()

N_CORES = 8
N_FULL = 32
IMG = N_FULL // N_CORES  # images per core
CIN = 128
COUT = 256
H = W = 64
HW = H * W
KDIM = 3
NTAP = KDIM * KDIM
CDIM = 64
XROWS = H + 2  # 2 zero rows for the H halo
CO_TILES = COUT // 128
ROWS_PER_BLK = 8
NBLK = H // ROWS_PER_BLK
BLK_N = ROWS_PER_BLK * W  # 512 = one fp32 PSUM bank
F32 = mybir.dt.float32
F32R = mybir.dt.float32r
BF16 = mybir.dt.bfloat16
WCOLS = CO_TILES * NTAP * 128  # co-tile-major weight layout

_cached_nc = None


def _build():
    nc = bacc.Bacc(
        "TRN2",
        target_bir_lowering=False,
        debug=False,
        enable_asserts=False,
        num_devices=N_CORES,
    )
    # x is host-baked into its exact SBUF layout per image: a leading zero
    # guard element, then XROWS rows of stride PWS=W+1 (top/bottom zero halo
    # rows included, zero guard column after each row) — so every image
    # loads with plain contiguous column-slice DMAs and no zero-fill pieces
    XCOLS = 1 + XROWS * (W + 1)
    x_d = nc.dram_tensor("x", (IMG, CIN, XCOLS), BF16, kind="ExternalInput").ap()
    wt_d = nc.dram_tensor("wt", (CIN, WCOLS), BF16, kind="ExternalInput").ap()
    # c/ones rows and c_weight/bias columns merged: [:, :COUT] is
    # [c_weight.T; bias], [:, COUT:] is [c.T; ones]
    cwbc_d = nc.dram_tensor(
        "cwbc", (CDIM + 1, COUT + IMG), BF16, kind="ExternalInput"
    ).ap()
    out_d = nc.dram_tensor("out", (IMG, COUT, H, W), F32, kind="ExternalOutput").ap()

    with tile.TileContext(nc) as tc:
        with (
            tc.tile_pool(name="consts", bufs=1) as consts,
            tc.tile_pool(name="xbuf", bufs=1) as xbuf,
            tc.tile_pool(name="obuf", bufs=2) as obuf,
            tc.tile_pool(name="ps", bufs=5, space="PSUM") as pspool,
            tc.tile_pool(name="cps", bufs=1, space="PSUM") as cpspool,
            tc.tile_pool(name="wps", bufs=1, space="PSUM") as wpspool,
        ):
            # PE warmup: the HAM clock gate needs ~3.4us of sustained matmul
            # activity to lift the 1.2GHz cold throttle, and the first real
            # matmul waits ~2us on the weight/image DMAs. 6 dummy N=512
            # matmuls (~2.6us cold) bridge the gap without delaying the
            # first real matmul behind a long warmup queue.
            warm_sb = consts.tile([CIN, BLK_N], BF16)
            nc.gpsimd.memset(warm_sb[:], 0.0)
            wps = wpspool.tile([128, BLK_N], F32)
            for _ in range(5):
                nc.tensor.matmul(
                    wps[:],
                    lhsT=warm_sb[:, 0:128],
                    rhs=warm_sb[:],
                    start=True,
                    stop=True,
                )

            # per-image input planes with stride-65 rows: position
            # 1 + u*PWS + c holds image pixel (u-1, c); column PWS-1 of each
            # row is a zero guard (baked into the host-padded x tensor), and
            # rows 0 / XROWS-1 plus the leading element are zeroed from z_d.
            # The +-1-column taps then read straight through the guards
            # (which contribute zero), so every tap is a uniform N=512
            # matmul with inner-contiguous rhs and a plain 2D PSUM out.
            PWS = W + 1

            def load_image(n, ring, cuts):
                """Emit the image-n load in len(cuts) column-slice pieces
                (cuts are exclusive xp-row upper bounds; the last must be
                XROWS). The host tensor already carries the halo/guards."""
                # one extra row of slack: tap AP slices extend past the last
                # guard before the [:, :, :W] crop trims them
                xp = xbuf.tile([CIN, 1 + (XROWS + 1) * PWS], BF16, tag=f"ximg{n}")
                a = 0
                for u in cuts:
                    b = 1 + u * PWS
                    ring.dma_start(out=xp[:, a:b], in_=x_d[n][:, a:b])
                    a = b
                return xp

            # image 0 leads the sync ring in fine pieces: rows 0-8 (plus
            # the top halo row) unblock conv block 0 as early as possible,
            # the next 8 rows block 1, and so on, while the weights stream
            # on the scalar ring. The merged context tensor rides between
            # the early pieces — the ctx matmuls run in the pipeline bubble
            # after conv block 0, so it isn't on the critical path.
            xp0 = xbuf.tile([CIN, 1 + (XROWS + 1) * PWS], BF16, tag="ximg0")
            nc.sync.dma_start(out=xp0[:, 0 : 1 + 10 * PWS], in_=x_d[0][:, 0 : 1 + 10 * PWS])
            nc.sync.dma_start(
                out=xp0[:, 1 + 10 * PWS : 1 + 18 * PWS],
                in_=x_d[0][:, 1 + 10 * PWS : 1 + 18 * PWS],
            )
            cwbc_sb = consts.tile([CDIM + 1, COUT + IMG], BF16)
            nc.sync.dma_start(out=cwbc_sb[:], in_=cwbc_d)
            for a, b in ((18, 34), (34, 50), (50, XROWS)):
                nc.sync.dma_start(
                    out=xp0[:, 1 + a * PWS : 1 + b * PWS],
                    in_=x_d[0][:, 1 + a * PWS : 1 + b * PWS],
                )
            xflats = {0: xp0}

            # co-tile-major bf16 weights on the scalar ring: co-tile 0 in
            # three tap-group pieces so block-0 matmuls start as soon as
            # their taps land, co-tile 1 (needed ~16us later) in one piece
            w_sb = consts.tile([CIN, WCOLS], BF16)
            TG = NTAP * 128  # one co-tile's weight columns
            for c0, c1 in ((0, 384), (384, 768), (768, TG), (TG, WCOLS)):
                nc.scalar.dma_start(out=w_sb[:, c0:c1], in_=wt_d[:, c0:c1])

            # ctxb[t][co, n] = sum_d c_weight[co, d] * c[n, d] + bias[co];
            # emitted after conv block 0's matmuls so the conv start isn't
            # gated on the cwbc DMA (the first epilogue needs ctxb ~1us
            # after block 0 finishes — plenty)
            ctxb = []

            def emit_ctx():
                for t in range(CO_TILES):
                    cps = cpspool.tile([128, IMG], F32, tag=f"cps{t}")
                    nc.tensor.matmul(
                        cps[:],
                        lhsT=cwbc_sb[:, t * 128 : (t + 1) * 128],
                        rhs=cwbc_sb[:, COUT : COUT + IMG],
                        start=True,
                        stop=True,
                    )
                    csb = consts.tile([128, IMG], F32, tag=f"ctxb{t}")
                    nc.vector.tensor_copy(csb[:], cps[:])
                    ctxb.append(csb)

            for n in range(IMG):
                xf = xflats[n]
                for t in range(CO_TILES):
                    obig = obuf.tile([128, HW], F32)
                    for b in range(NBLK):
                        ps = pspool.tile([128, BLK_N], F32)
                        r0 = b * ROWS_PER_BLK
                        for i in range(NTAP):
                            kh, kw = divmod(i, KDIM)
                            w0 = (t * NTAP + i) * 128
                            o = 1 + (r0 + kh) * PWS + (kw - 1)
                            rhs = xf[:, o : o + ROWS_PER_BLK * PWS].rearrange(
                                "p (r c) -> p r c", c=PWS
                            )[:, :, :W]
                            nc.tensor.matmul(
                                ps[:],
                                lhsT=w_sb[:, w0 : w0 + 128],
                                rhs=rhs,
                                start=(i == 0),
                                stop=(i == NTAP - 1),
                            )
                        if n == 0 and t == 0 and b == 0:
                            emit_ctx()
                        oslice = obig[:, b * BLK_N : (b + 1) * BLK_N]
                        last_blk = (
                            n == IMG - 1 and t == CO_TILES - 1 and b == NBLK - 1
                        )
                        if last_blk:
                            # split the final eviction across DVE and ACT so
                            # both halves of the tail store launch at once
                            hb = BLK_N // 2
                            nc.vector.tensor_scalar_add(
                                oslice[:, 0:hb], ps[:, 0:hb], ctxb[t][:, n : n + 1]
                            )
                            nc.scalar.activation(
                                oslice[:, hb:BLK_N],
                                ps[:, hb:BLK_N],
                                mybir.ActivationFunctionType.Identity,
                                bias=ctxb[t][:, n : n + 1],
                                scale=1.0,
                            )
                        elif t == 0:
                            nc.scalar.activation(
                                oslice,
                                ps[:],
                                mybir.ActivationFunctionType.Identity,
                                bias=ctxb[t][:, n : n + 1],
                                scale=1.0,
                            )
                        else:
                            nc.vector.tensor_scalar_add(
                                oslice, ps[:], ctxb[t][:, n : n + 1]
                            )
                    # split the 2MB plane store so the last piece doesn't sit
                    # whole on the kernel's critical tail; the very last
                    # plane goes in 8 pieces with the final 256KB split
                    # across both rings
                    oflat = out_d[n, t * 128 : (t + 1) * 128].rearrange(
                        "o h w -> o (h w)"
                    )
                    if n == IMG - 1 and t == CO_TILES - 1:
                        P8 = HW // 8
                        for q in range(7):
                            nc.sync.dma_start(
                                out=oflat[:, q * P8 : (q + 1) * P8],
                                in_=obig[:, q * P8 : (q + 1) * P8],
                            )
                        nc.sync.dma_start(
                            out=oflat[:, 7 * P8 : 7 * P8 + P8 // 2],
                            in_=obig[:, 7 * P8 : 7 * P8 + P8 // 2],
                        )
                        nc.scalar.dma_start(
                            out=oflat[:, 7 * P8 + P8 // 2 : HW],
                            in_=obig[:, 7 * P8 + P8 // 2 : HW],
                        )
                    else:
                        for q in range(4):
                            nc.sync.dma_start(
                                out=oflat[:, q * (HW // 4) : (q + 1) * (HW // 4)],
                                in_=obig[:, q * (HW // 4) : (q + 1) * (HW // 4)],
                            )
                    # prefetch the next image while this one's second
                    # C_out tile computes
                    if t == 0 and n + 1 < IMG:
                        xflats[n + 1] = load_image(n + 1, nc.scalar, (XROWS,))
    nc.compile()
    return nc


def get_nc():
    global _cached_nc
    if _cached_nc is None:
        _cached_nc = _build()
    return _cached_nc


def prep_in_maps(x, c, weight, c_weight, bias):
    import ml_dtypes

    bf16 = ml_dtypes.bfloat16
    x = np.ascontiguousarray(np.asarray(x, dtype=np.float32))
    c = np.asarray(c, dtype=np.float32)
    weight = np.asarray(weight, dtype=np.float32)
    c_weight = np.asarray(c_weight, dtype=np.float32)
    bias = np.asarray(bias, dtype=np.float32)

    # co-tile-major: wt[ci, t*NTAP*128 + i*128 + co] = weight[t*128+co, ci, i]
    wt = np.ascontiguousarray(
        weight.reshape(CO_TILES, 128, CIN, NTAP)
        .transpose(2, 0, 3, 1)
        .reshape(CIN, WCOLS)
        .astype(bf16)
    )
    cwb = np.concatenate([c_weight.T, bias[None, :]], axis=0)
    # host-baked SBUF image layout: leading zero guard element, XROWS rows
    # of stride W+1 with zero top/bottom halo rows and zero guard columns
    PWS = W + 1
    XCOLS = 1 + XROWS * PWS
    xbig = np.zeros((N_FULL, CIN, XCOLS), bf16)
    xbig[:, :, 1 + PWS : 1 + PWS + H * PWS].reshape(N_FULL, CIN, H, PWS)[
        :, :, :, :W
    ] = x
    in_maps = []
    for i in range(N_CORES):
        xs = np.ascontiguousarray(xbig[i * IMG : (i + 1) * IMG])
        cb = np.concatenate(
            [c[i * IMG : (i + 1) * IMG].T, np.ones((1, IMG), np.float32)], axis=0
        )
        cwbc = np.ascontiguousarray(
            np.concatenate([cwb, cb], axis=1).astype(bf16)
        )
        in_maps.append({"x": xs, "wt": wt, "cwbc": cwbc})
    return in_maps


def run(x, c, weight, c_weight, bias, trace=False):
    nc = get_nc()
    in_maps = prep_in_maps(x, c, weight, c_weight, bias)
    last_err = None
    for attempt in range(3):
        try:
            res = bass_utils.run_bass_kernel_spmd(
                nc, in_maps, core_ids=list(range(N_CORES)), trace=trace
            )
            break
        except Exception as e:  # noqa: BLE001
            # NRT_EXEC_UNIT_UNRECOVERABLE occasionally fires spuriously;
            # a reloaded execution recovers
            last_err = e
            time.sleep(2.0)
    else:
        raise last_err
    out = np.concatenate([res.results[i]["out"] for i in range(N_CORES)], axis=0)
    return out, res


def kernel(x, c, weight, c_weight, bias):
    out, _ = run(x, c, weight, c_weight, bias)
    return out


# revision 18
# speedup vs baseline: 1.0058x; 1.0058x over previous
"""ContextualConv2d Trainium2 kernel.

out = conv2d(x, weight, pad=1) + (c @ c_weight.T)[:, :, None, None] + bias[None, :, None, None]

Full shapes: x (32,128,64,64) f32, c (32,64), weight (256,128,3,3),
c_weight (256,64), bias (256,) -> out (32,256,64,64).

Strategy: data-parallel over batch across 8 NeuronCores (4 images each).
Per core the conv is an implicit GEMM: each image lives in SBUF with
stride-65 rows (a host-baked zero guard column after each 64-pixel row,
plus two zero rows for the H halo), so the +-1-column filter taps read
straight through zero guards and every tap is a uniform N=512 matmul
with inner-contiguous rhs. For each 128-wide C_out tile and each
512-column output block (8 image rows x 64 cols), 9 matmuls (one per
filter tap) accumulate into a PSUM bank.

Conv operands are bf16 (~3e-3 rel err, well under the 2e-2 gate): bf16
moving streams at the full 1 elem/cycle PE rate with a ~216ns warm
back-to-back gap at N=512, vs ~237ns measured for fp32r (fp32-class
LDWEIGHTS is slower and only partially hidden), and input DMA bytes
halve. Weights are stored co-tile-major (t, tap, 128) and DMAed in two
halves so the first conv matmul only waits on the first 0.3MB; image 0
leads the sync ring in 3 interior pieces while the weights ride the
scalar ring in parallel. The context bias (c @ c_weight.T + bias) comes
from one small on-device fp32r matmul per C_out tile (a ones-row on the
rhs folds in the channel bias) and is fused into the PSUM->SBUF
epilogue on ACT (co-tile 0) / DVE (co-tile 1).

Schedule: 6 bf16 warmup matmuls keep the PE busy (HAM un-throttle)
while the first inputs stream; images 1-3 are prefetched one compute
pass ahead on the scalar ring; output planes are stored in 4 x 512KB
contiguous pieces, except the last plane which goes in 8 x 256KB pieces
with the final 256KB split across both rings so the kernel tail only
carries ~128KB of store. Baseline (fp32r, serial weight wait): ~159us.
"""

import sys
import time
import types

import numpy as np

import concourse.tile as tile
from concourse import bacc, bass_utils, mybir


def _ensure_axon_hooks_shim():
    """concourse imports antenv.axon_hooks when BASS_TRACE is set; the agent
    image's antenv lacks it. Provide a null shim so tracing degrades to a
    warning instead of an ImportError."""
    try:
        import antenv

        if not hasattr(antenv, "axon_hooks"):
            try:
                from antenv import axon_hooks  # noqa: F401
            except ImportError:
                mod = types.ModuleType("antenv.axon_hooks")
                _state = {"hook": None}
                mod.set_axon_ntff_profile_hook = lambda h: _state.__setitem__(
                    "hook", h
                )
                mod.get_axon_ntff_profile_hook = lambda: _state["hook"]
                sys.modules["antenv.axon_hooks"] = mod
                antenv.axon_hooks = mod
    except Exception:
        pass


_ensure_axon_hooks_shim()

N_CORES = 8
N_FULL = 32
IMG = N_FULL // N_CORES  # images per core
CIN = 128
COUT = 256
H = W = 64
HW = H * W
KDIM = 3
NTAP = KDIM * KDIM
CDIM = 64
XROWS = H + 2  # 2 zero rows for the H halo
CO_TILES = COUT // 128
ROWS_PER_BLK = 8
NBLK = H // ROWS_PER_BLK
BLK_N = ROWS_PER_BLK * W  # 512 = one fp32 PSUM bank
F32 = mybir.dt.float32
F32R = mybir.dt.float32r
BF16 = mybir.dt.bfloat16
WCOLS = CO_TILES * NTAP * 128  # co-tile-major weight layout

_cached_nc = None


def _build():
    nc = bacc.Bacc(
        "TRN2",
        target_bir_lowering=False,
        debug=False,
        enable_asserts=False,
        num_devices=N_CORES,
    )
    # x is host-baked into its exact SBUF layout per image: a leading zero
    # guard element, then XROWS rows of stride PWS=W+1 (top/bottom zero halo
    # rows included, zero guard column after each row) — so every image
    # loads with plain contiguous column-slice DMAs and no zero-fill pieces
    XCOLS = 1 + XROWS * (W + 1)
    x_d = nc.dram_tensor("x", (IMG, CIN, XCOLS), BF16, kind="ExternalInput").ap()
    wt_d = nc.dram_tensor("wt", (CIN, WCOLS), BF16, kind="ExternalInput").ap()
    # c/ones rows and c_weight/bias columns merged: [:, :COUT] is
    # [c_weight.T; bias], [:, COUT:] is [c.T; ones]
    cwbc_d = nc.dram_tensor(
        "cwbc", (CDIM + 1, COUT + IMG), BF16, kind="ExternalInput"
    ).ap()
    out_d = nc.dram_tensor("out", (IMG, COUT, H, W), F32, kind="ExternalOutput").ap()

    with tile.TileContext(nc) as tc:
        with (
            tc.tile_pool(name="consts", bufs=1) as consts,
            tc.tile_pool(name="xbuf", bufs=1) as xbuf,
            tc.tile_pool(name="obuf", bufs=2) as obuf,
            tc.tile_pool(name="ps", bufs=5, space="PSUM") as pspool,
            tc.tile_pool(name="cps", bufs=1, space="PSUM") as cpspool,
            tc.tile_pool(name="wps", bufs=1, space="PSUM") as wpspool,
        ):
            # PE warmup: the HAM clock gate needs ~3.4us of sustained matmul
            # activity to lift the 1.2GHz cold throttle, and the first real
            # matmul waits ~2us on the weight/image DMAs. 6 dummy N=512
            # matmuls (~2.6us cold) bridge the gap without delaying the
            # first real matmul behind a long warmup queue.
            warm_sb = consts.tile([CIN, BLK_N], BF16)
            nc.gpsimd.memset(warm_sb[:], 0.0)
            wps = wpspool.tile([128, BLK_N], F32)
            for _ in range(5):
                nc.tensor.matmul(
                    wps[:],
                    lhsT=warm_sb[:, 0:128],
                    rhs=warm_sb[:],
                    start=True,
                    stop=True,
                )

            # per-image input planes with stride-65 rows: position
            # 1 + u*PWS + c holds image pixel (u-1, c); column PWS-1 of each
            # row is a zero guard (baked into the host-padded x tensor), and
            # rows 0 / XROWS-1 plus the leading element are zeroed from z_d.
            # The +-1-column taps then read straight through the guards
            # (which contribute zero), so every tap is a uniform N=512
            # matmul with inner-contiguous rhs and a plain 2D PSUM out.
            PWS = W + 1

            def load_image(n, ring, cuts):
                """Emit the image-n load in len(cuts) column-slice pieces
                (cuts are exclusive xp-row upper bounds; the last must be
                XROWS). The host tensor already carries the halo/guards."""
                # one extra row of slack: tap AP slices extend past the last
                # guard before the [:, :, :W] crop trims them
                xp = xbuf.tile([CIN, 1 + (XROWS + 1) * PWS], BF16, tag=f"ximg{n}")
                a = 0
                for u in cuts:
                    b = 1 + u * PWS
                    ring.dma_start(out=xp[:, a:b], in_=x_d[n][:, a:b])
                    a = b
                return xp

            # Each DMA piece costs ~1.3us of fixed latency on its ring and
            # the rings deliver only ~100-110GB/s early, so image 0 is
            # interleaved across BOTH rings in 8-row pieces — each ring
            # supplies every other conv block and neither falls behind the
            # ~2us/block consumption rate. Weights lead the scalar ring in
            # three tap-group pieces (block-0 matmuls consume taps in
            # order); co-tile 1 rides later (needed only at ~27us).
            xp0 = xbuf.tile([CIN, 1 + (XROWS + 1) * PWS], BF16, tag="ximg0")
            xflats = {0: xp0}

            def x0_piece(ring, a, b):
                lo = 0 if a == 0 else 1 + a * PWS
                hi = 1 + b * PWS
                ring.dma_start(out=xp0[:, lo:hi], in_=x_d[0][:, lo:hi])

            w_sb = consts.tile([CIN, WCOLS], BF16)
            TG = NTAP * 128  # one co-tile's weight columns
            cwbc_sb = consts.tile([CDIM + 1, COUT + IMG], BF16)

            for c0, c1 in ((0, 384), (384, 768), (768, TG)):
                nc.scalar.dma_start(out=w_sb[:, c0:c1], in_=wt_d[:, c0:c1])
            x0_piece(nc.sync, 0, 10)
            x0_piece(nc.sync, 10, 18)
            x0_piece(nc.sync, 18, 26)
            nc.scalar.dma_start(out=cwbc_sb[:], in_=cwbc_d)
            x0_piece(nc.scalar, 26, 34)
            x0_piece(nc.sync, 34, 42)
            nc.scalar.dma_start(out=w_sb[:, TG:WCOLS], in_=wt_d[:, TG:WCOLS])
            x0_piece(nc.scalar, 42, 50)
            x0_piece(nc.sync, 50, 58)
            x0_piece(nc.scalar, 58, XROWS)

            # ctxb[t][co, n] = sum_d c_weight[co, d] * c[n, d] + bias[co];
            # emitted after conv block 0's matmuls so the conv start isn't
            # gated on the cwbc DMA (the first epilogue needs ctxb ~1us
            # after block 0 finishes — plenty)
            ctxb = []

            def emit_ctx():
                for t in range(CO_TILES):
                    cps = cpspool.tile([128, IMG], F32, tag=f"cps{t}")
                    nc.tensor.matmul(
                        cps[:],
                        lhsT=cwbc_sb[:, t * 128 : (t + 1) * 128],
                        rhs=cwbc_sb[:, COUT : COUT + IMG],
                        start=True,
                        stop=True,
                    )
                    csb = consts.tile([128, IMG], F32, tag=f"ctxb{t}")
                    nc.vector.tensor_copy(csb[:], cps[:])
                    ctxb.append(csb)

            for n in range(IMG):
                xf = xflats[n]
                for t in range(CO_TILES):
                    obig = obuf.tile([128, HW], F32)
                    pending = None
                    for b in range(NBLK):
                        ps = pspool.tile([128, BLK_N], F32)
                        r0 = b * ROWS_PER_BLK
                        for i in range(NTAP):
                            kh, kw = divmod(i, KDIM)
                            w0 = (t * NTAP + i) * 128
                            o = 1 + (r0 + kh) * PWS + (kw - 1)
                            rhs = xf[:, o : o + ROWS_PER_BLK * PWS].rearrange(
                                "p (r c) -> p r c", c=PWS
                            )[:, :, :W]
                            nc.tensor.matmul(
                                ps[:],
                                lhsT=w_sb[:, w0 : w0 + 128],
                                rhs=rhs,
                                start=(i == 0),
                                stop=(i == NTAP - 1),
                            )
                        if n == 0 and t == 0:
                            # the ctx matmuls sit between block 1 and block 2
                            # in the tensor queue, giving the cwbc DMA until
                            # ~14us to land; block 0's epilogue (which needs
                            # ctxb) is deferred until after they're emitted
                            if b == 0:
                                pending = ps
                                continue
                            if b == 1:
                                emit_ctx()
                                nc.scalar.activation(
                                    obig[:, 0:BLK_N],
                                    pending[:],
                                    mybir.ActivationFunctionType.Identity,
                                    bias=ctxb[0][:, 0:1],
                                    scale=1.0,
                                )
                        oslice = obig[:, b * BLK_N : (b + 1) * BLK_N]
                        last_blk = (
                            n == IMG - 1 and t == CO_TILES - 1 and b == NBLK - 1
                        )
                        if last_blk:
                            # split the final eviction across DVE and ACT so
                            # both halves of the tail store launch at once
                            hb = BLK_N // 2
                            nc.vector.tensor_scalar_add(
                                oslice[:, 0:hb], ps[:, 0:hb], ctxb[t][:, n : n + 1]
                            )
                            nc.scalar.activation(
                                oslice[:, hb:BLK_N],
                                ps[:, hb:BLK_N],
                                mybir.ActivationFunctionType.Identity,
                                bias=ctxb[t][:, n : n + 1],
                                scale=1.0,
                            )
                        elif t == 0:
                            nc.scalar.activation(
                                oslice,
                                ps[:],
                                mybir.ActivationFunctionType.Identity,
                                bias=ctxb[t][:, n : n + 1],
                                scale=1.0,
                            )
                        else:
                            nc.vector.tensor_scalar_add(
                                oslice, ps[:], ctxb[t][:, n : n + 1]
                            )
                    # split the 2MB plane store so the last piece doesn't sit
                    # whole on the kernel's critical tail; the very last
                    # plane goes in 8 pieces with the final 256KB split
                    # across both rings
                    oflat = out_d[n, t * 128 : (t + 1) * 128].rearrange(
                        "o h w -> o (h w)"
                    )
                    if n == IMG - 1 and t == CO_TILES - 1:
                        P8 = HW // 8
                        for q in range(7):
                            nc.sync.dma_start(
                                out=oflat[:, q * P8 : (q + 1) * P8],
                                in_=obig[:, q * P8 : (q + 1) * P8],
                            )
                        nc.sync.dma_start(
                            out=oflat[:, 7 * P8 : 7 * P8 + P8 // 2],
                            in_=obig[:, 7 * P8 : 7 * P8 + P8 // 2],
                        )
                        nc.scalar.dma_start(
                            out=oflat[:, 7 * P8 + P8 // 2 : HW],
                            in_=obig[:, 7 * P8 + P8 // 2 : HW],
                        )
                    else:
                        for q in range(4):
                            nc.sync.dma_start(
                                out=oflat[:, q * (HW // 4) : (q + 1) * (HW // 4)],
                                in_=obig[:, q * (HW // 4) : (q + 1) * (HW // 4)],
                            )
                    # prefetch the next image while this one's second
                    # C_out tile computes
                    if t == 0 and n + 1 < IMG:
                        xflats[n + 1] = load_image(n + 1, nc.scalar, (XROWS,))
    nc.compile()
    return nc


def get_nc():
    global _cached_nc
    if _cached_nc is None:
        _cached_nc = _build()
    return _cached_nc


def prep_in_maps(x, c, weight, c_weight, bias):
    import ml_dtypes

    bf16 = ml_dtypes.bfloat16
    x = np.ascontiguousarray(np.asarray(x, dtype=np.float32))
    c = np.asarray(c, dtype=np.float32)
    weight = np.asarray(weight, dtype=np.float32)
    c_weight = np.asarray(c_weight, dtype=np.float32)
    bias = np.asarray(bias, dtype=np.float32)

    # co-tile-major: wt[ci, t*NTAP*128 + i*128 + co] = weight[t*128+co, ci, i]
    wt = np.ascontiguousarray(
        weight.reshape(CO_TILES, 128, CIN, NTAP)
        .transpose(2, 0, 3, 1)
        .reshape(CIN, WCOLS)
        .astype(bf16)
    )
    cwb = np.concatenate([c_weight.T, bias[None, :]], axis=0)
    # host-baked SBUF image layout: leading zero guard element, XROWS rows
    # of stride W+1 with zero top/bottom halo rows and zero guard columns
    PWS = W + 1
    XCOLS = 1 + XROWS * PWS
    xbig = np.zeros((N_FULL, CIN, XCOLS), bf16)
    xbig[:, :, 1 + PWS : 1 + PWS + H * PWS].reshape(N_FULL, CIN, H, PWS)[
        :, :, :, :W
    ] = x
    in_maps = []
    for i in range(N_CORES):
        xs = np.ascontiguousarray(xbig[i * IMG : (i + 1) * IMG])
        cb = np.concatenate(
            [c[i * IMG : (i + 1) * IMG].T, np.ones((1, IMG), np.float32)], axis=0
        )
        cwbc = np.ascontiguousarray(
            np.concatenate([cwb, cb], axis=1).astype(bf16)
        )
        in_maps.append({"x": xs, "wt": wt, "cwbc": cwbc})
    return in_maps


def run(x, c, weight, c_weight, bias, trace=False):
    nc = get_nc()
    in_maps = prep_in_maps(x, c, weight, c_weight, bias)
    last_err = None
    for attempt in range(3):
        try:
            res = bass_utils.run_bass_kernel_spmd(
                nc, in_maps, core_ids=list(range(N_CORES)), trace=trace
            )
            break
        except Exception as e:  # noqa: BLE001
            # NRT_EXEC_UNIT_UNRECOVERABLE occasionally fires spuriously;
            # a reloaded execution recovers
            last_err = e
            time.sleep(2.0)
    else:
        raise last_err
    out = np.concatenate([res.results[i]["out"] for i in range(N_CORES)], axis=0)
    return out, res


def kernel(x, c, weight, c_weight, bias):
    out, _ = run(x, c, weight, c_weight, bias)
    return out


# revision 19
# speedup vs baseline: 1.0143x; 1.0084x over previous
"""ContextualConv2d Trainium2 kernel.

out = conv2d(x, weight, pad=1) + (c @ c_weight.T)[:, :, None, None] + bias[None, :, None, None]

Full shapes: x (32,128,64,64) f32, c (32,64), weight (256,128,3,3),
c_weight (256,64), bias (256,) -> out (32,256,64,64).

Strategy: data-parallel over batch across 8 NeuronCores (4 images each).
Per core the conv is an implicit GEMM: each image lives in SBUF with
stride-65 rows (a host-baked zero guard column after each 64-pixel row,
plus two zero rows for the H halo), so the +-1-column filter taps read
straight through zero guards and every tap is a uniform N=512 matmul
with inner-contiguous rhs. For each 128-wide C_out tile and each
512-column output block (8 image rows x 64 cols), 9 matmuls (one per
filter tap) accumulate into a PSUM bank.

Conv operands are bf16 (~3e-3 rel err, well under the 2e-2 gate): bf16
moving streams at the full 1 elem/cycle PE rate with a ~216ns warm
back-to-back gap at N=512, vs ~237ns measured for fp32r (fp32-class
LDWEIGHTS is slower and only partially hidden), and input DMA bytes
halve. Weights are stored co-tile-major (t, tap, 128) and DMAed in two
halves so the first conv matmul only waits on the first 0.3MB; image 0
leads the sync ring in 3 interior pieces while the weights ride the
scalar ring in parallel. The context bias (c @ c_weight.T + bias) comes
from one small on-device fp32r matmul per C_out tile (a ones-row on the
rhs folds in the channel bias) and is fused into the PSUM->SBUF
epilogue on ACT (co-tile 0) / DVE (co-tile 1).

Schedule: 6 bf16 warmup matmuls keep the PE busy (HAM un-throttle)
while the first inputs stream; images 1-3 are prefetched one compute
pass ahead on the scalar ring; output planes are stored in 4 x 512KB
contiguous pieces, except the last plane which goes in 8 x 256KB pieces
with the final 256KB split across both rings so the kernel tail only
carries ~128KB of store. Baseline (fp32r, serial weight wait): ~159us.
"""

import sys
import time
import types

import numpy as np

import concourse.tile as tile
from concourse import bacc, bass_utils, mybir


def _ensure_axon_hooks_shim():
    """concourse imports antenv.axon_hooks when BASS_TRACE is set; the agent
    image's antenv lacks it. Provide a null shim so tracing degrades to a
    warning instead of an ImportError."""
    try:
        import antenv

        if not hasattr(antenv, "axon_hooks"):
            try:
                from antenv import axon_hooks  # noqa: F401
            except ImportError:
                mod = types.ModuleType("antenv.axon_hooks")
                _state = {"hook": None}
                mod.set_axon_ntff_profile_hook = lambda h: _state.__setitem__(
                    "hook", h
                )
                mod.get_axon_ntff_profile_hook = lambda: _state["hook"]
                sys.modules["antenv.axon_hooks"] = mod
                antenv.axon_hooks = mod
    except Exception:
        pass


_ensure_axon_hooks_shim()

N_CORES = 8
N_FULL = 32
IMG = N_FULL // N_CORES  # images per core
CIN = 128
COUT = 256
H = W = 64
HW = H * W
KDIM = 3
NTAP = KDIM * KDIM
CDIM = 64
XROWS = H + 2  # 2 zero rows for the H halo
CO_TILES = COUT // 128
ROWS_PER_BLK = 8
NBLK = H // ROWS_PER_BLK
BLK_N = ROWS_PER_BLK * W  # 512 = one fp32 PSUM bank
F32 = mybir.dt.float32
F32R = mybir.dt.float32r
BF16 = mybir.dt.bfloat16
WCOLS = CO_TILES * NTAP * 128  # co-tile-major weight layout

_cached_nc = None


def _build():
    nc = bacc.Bacc(
        "TRN2",
        target_bir_lowering=False,
        debug=False,
        enable_asserts=False,
        num_devices=N_CORES,
    )
    # x is host-baked into its exact SBUF layout per image: a leading zero
    # guard element, then XROWS rows of stride PWS=W+1 (top/bottom zero halo
    # rows included, zero guard column after each row) — so every image
    # loads with plain contiguous column-slice DMAs and no zero-fill pieces
    XCOLS = 1 + XROWS * (W + 1)
    x_d = nc.dram_tensor("x", (IMG, CIN, XCOLS), BF16, kind="ExternalInput").ap()
    wt_d = nc.dram_tensor("wt", (CIN, WCOLS), BF16, kind="ExternalInput").ap()
    # c/ones rows and c_weight/bias columns merged: [:, :COUT] is
    # [c_weight.T; bias], [:, COUT:] is [c.T; ones]
    cwbc_d = nc.dram_tensor(
        "cwbc", (CDIM + 1, COUT + IMG), BF16, kind="ExternalInput"
    ).ap()
    out_d = nc.dram_tensor("out", (IMG, COUT, H, W), F32, kind="ExternalOutput").ap()

    with tile.TileContext(nc) as tc:
        with (
            tc.tile_pool(name="consts", bufs=1) as consts,
            tc.tile_pool(name="xbuf", bufs=1) as xbuf,
            tc.tile_pool(name="obuf", bufs=2) as obuf,
            tc.tile_pool(name="ps", bufs=5, space="PSUM") as pspool,
            tc.tile_pool(name="cps", bufs=1, space="PSUM") as cpspool,
            tc.tile_pool(name="wps", bufs=1, space="PSUM") as wpspool,
        ):
            # PE warmup: the HAM clock gate needs ~3.4us of sustained matmul
            # activity to lift the 1.2GHz cold throttle, and any idle gap
            # between warmup and the first real matmul restarts the busy
            # window (costing ~2us of cold conv matmuls). 13 N=256 dummy
            # matmuls (~213ns each cold) bridge from the ~7.7us preamble end
            # to the ~10.5us conv start with fine granularity, so the
            # handoff gap stays under one matmul.
            warm_sb = consts.tile([CIN, BLK_N], BF16)
            nc.gpsimd.memset(warm_sb[:], 0.0)
            wps = wpspool.tile([128, BLK_N], F32)
            for _ in range(13):
                nc.tensor.matmul(
                    wps[:, 0 : BLK_N // 2],
                    lhsT=warm_sb[:, 0:128],
                    rhs=warm_sb[:, 0 : BLK_N // 2],
                    start=True,
                    stop=True,
                )

            # per-image input planes with stride-65 rows: position
            # 1 + u*PWS + c holds image pixel (u-1, c); column PWS-1 of each
            # row is a zero guard (baked into the host-padded x tensor), and
            # rows 0 / XROWS-1 plus the leading element are zeroed from z_d.
            # The +-1-column taps then read straight through the guards
            # (which contribute zero), so every tap is a uniform N=512
            # matmul with inner-contiguous rhs and a plain 2D PSUM out.
            PWS = W + 1

            def load_image(n, ring, cuts):
                """Emit the image-n load in len(cuts) column-slice pieces
                (cuts are exclusive xp-row upper bounds; the last must be
                XROWS). The host tensor already carries the halo/guards."""
                # one extra row of slack: tap AP slices extend past the last
                # guard before the [:, :, :W] crop trims them
                xp = xbuf.tile([CIN, 1 + (XROWS + 1) * PWS], BF16, tag=f"ximg{n}")
                a = 0
                for u in cuts:
                    b = 1 + u * PWS
                    ring.dma_start(out=xp[:, a:b], in_=x_d[n][:, a:b])
                    a = b
                return xp

            # Each DMA piece costs ~1.3us of fixed latency on its ring and
            # the rings deliver only ~100-110GB/s early, so image 0 is
            # interleaved across BOTH rings in 8-row pieces — each ring
            # supplies every other conv block and neither falls behind the
            # ~2us/block consumption rate. Weights lead the scalar ring in
            # three tap-group pieces (block-0 matmuls consume taps in
            # order); co-tile 1 rides later (needed only at ~27us).
            xp0 = xbuf.tile([CIN, 1 + (XROWS + 1) * PWS], BF16, tag="ximg0")
            xflats = {0: xp0}

            def x0_piece(ring, a, b):
                lo = 0 if a == 0 else 1 + a * PWS
                hi = 1 + b * PWS
                ring.dma_start(out=xp0[:, lo:hi], in_=x_d[0][:, lo:hi])

            w_sb = consts.tile([CIN, WCOLS], BF16)
            TG = NTAP * 128  # one co-tile's weight columns
            cwbc_sb = consts.tile([CDIM + 1, COUT + IMG], BF16)

            for c0, c1 in ((0, 384), (384, 768), (768, TG)):
                nc.scalar.dma_start(out=w_sb[:, c0:c1], in_=wt_d[:, c0:c1])
            x0_piece(nc.sync, 0, 10)
            x0_piece(nc.sync, 10, 18)
            x0_piece(nc.sync, 18, 26)
            nc.scalar.dma_start(out=cwbc_sb[:], in_=cwbc_d)
            x0_piece(nc.scalar, 26, 34)
            x0_piece(nc.sync, 34, 42)
            nc.scalar.dma_start(out=w_sb[:, TG:WCOLS], in_=wt_d[:, TG:WCOLS])
            x0_piece(nc.scalar, 42, 50)
            x0_piece(nc.sync, 50, 58)
            x0_piece(nc.scalar, 58, XROWS)

            # ctxb[t][co, n] = sum_d c_weight[co, d] * c[n, d] + bias[co];
            # emitted after conv block 0's matmuls so the conv start isn't
            # gated on the cwbc DMA (the first epilogue needs ctxb ~1us
            # after block 0 finishes — plenty)
            ctxb = []

            def emit_ctx():
                for t in range(CO_TILES):
                    cps = cpspool.tile([128, IMG], F32, tag=f"cps{t}")
                    nc.tensor.matmul(
                        cps[:],
                        lhsT=cwbc_sb[:, t * 128 : (t + 1) * 128],
                        rhs=cwbc_sb[:, COUT : COUT + IMG],
                        start=True,
                        stop=True,
                    )
                    csb = consts.tile([128, IMG], F32, tag=f"ctxb{t}")
                    nc.vector.tensor_copy(csb[:], cps[:])
                    ctxb.append(csb)

            for n in range(IMG):
                xf = xflats[n]
                for t in range(CO_TILES):
                    obig = obuf.tile([128, HW], F32)
                    pending = None
                    for b in range(NBLK):
                        ps = pspool.tile([128, BLK_N], F32)
                        r0 = b * ROWS_PER_BLK
                        for i in range(NTAP):
                            kh, kw = divmod(i, KDIM)
                            w0 = (t * NTAP + i) * 128
                            o = 1 + (r0 + kh) * PWS + (kw - 1)
                            rhs = xf[:, o : o + ROWS_PER_BLK * PWS].rearrange(
                                "p (r c) -> p r c", c=PWS
                            )[:, :, :W]
                            nc.tensor.matmul(
                                ps[:],
                                lhsT=w_sb[:, w0 : w0 + 128],
                                rhs=rhs,
                                start=(i == 0),
                                stop=(i == NTAP - 1),
                            )
                        if n == 0 and t == 0:
                            # the ctx matmuls sit between block 1 and block 2
                            # in the tensor queue, giving the cwbc DMA until
                            # ~14us to land; block 0's epilogue (which needs
                            # ctxb) is deferred until after they're emitted
                            if b == 0:
                                pending = ps
                                continue
                            if b == 1:
                                emit_ctx()
                                nc.scalar.activation(
                                    obig[:, 0:BLK_N],
                                    pending[:],
                                    mybir.ActivationFunctionType.Identity,
                                    bias=ctxb[0][:, 0:1],
                                    scale=1.0,
                                )
                        oslice = obig[:, b * BLK_N : (b + 1) * BLK_N]
                        last_blk = (
                            n == IMG - 1 and t == CO_TILES - 1 and b == NBLK - 1
                        )
                        if last_blk:
                            # split the final eviction across DVE and ACT so
                            # both halves of the tail store launch at once
                            hb = BLK_N // 2
                            nc.vector.tensor_scalar_add(
                                oslice[:, 0:hb], ps[:, 0:hb], ctxb[t][:, n : n + 1]
                            )
                            nc.scalar.activation(
                                oslice[:, hb:BLK_N],
                                ps[:, hb:BLK_N],
                                mybir.ActivationFunctionType.Identity,
                                bias=ctxb[t][:, n : n + 1],
                                scale=1.0,
                            )
                        elif t == 0:
                            nc.scalar.activation(
                                oslice,
                                ps[:],
                                mybir.ActivationFunctionType.Identity,
                                bias=ctxb[t][:, n : n + 1],
                                scale=1.0,
                            )
                        else:
                            nc.vector.tensor_scalar_add(
                                oslice, ps[:], ctxb[t][:, n : n + 1]
                            )
                    # split the 2MB plane store so the last piece doesn't sit
                    # whole on the kernel's critical tail; the very last
                    # plane goes in 8 pieces with the final 256KB split
                    # across both rings
                    oflat = out_d[n, t * 128 : (t + 1) * 128].rearrange(
                        "o h w -> o (h w)"
                    )
                    if n == IMG - 1 and t == CO_TILES - 1:
                        P8 = HW // 8
                        for q in range(7):
                            nc.sync.dma_start(
                                out=oflat[:, q * P8 : (q + 1) * P8],
                                in_=obig[:, q * P8 : (q + 1) * P8],
                            )
                        nc.sync.dma_start(
                            out=oflat[:, 7 * P8 : 7 * P8 + P8 // 2],
                            in_=obig[:, 7 * P8 : 7 * P8 + P8 // 2],
                        )
                        nc.scalar.dma_start(
                            out=oflat[:, 7 * P8 + P8 // 2 : HW],
                            in_=obig[:, 7 * P8 + P8 // 2 : HW],
                        )
                    else:
                        for q in range(4):
                            nc.sync.dma_start(
                                out=oflat[:, q * (HW // 4) : (q + 1) * (HW // 4)],
                                in_=obig[:, q * (HW // 4) : (q + 1) * (HW // 4)],
                            )
                    # prefetch the next image while this one's second
                    # C_out tile computes
                    if t == 0 and n + 1 < IMG:
                        xflats[n + 1] = load_image(n + 1, nc.scalar, (XROWS,))
    nc.compile()
    return nc


def get_nc():
    global _cached_nc
    if _cached_nc is None:
        _cached_nc = _build()
    return _cached_nc


def prep_in_maps(x, c, weight, c_weight, bias):
    import ml_dtypes

    bf16 = ml_dtypes.bfloat16
    x = np.ascontiguousarray(np.asarray(x, dtype=np.float32))
    c = np.asarray(c, dtype=np.float32)
    weight = np.asarray(weight, dtype=np.float32)
    c_weight = np.asarray(c_weight, dtype=np.float32)
    bias = np.asarray(bias, dtype=np.float32)

    # co-tile-major: wt[ci, t*NTAP*128 + i*128 + co] = weight[t*128+co, ci, i]
    wt = np.ascontiguousarray(
        weight.reshape(CO_TILES, 128, CIN, NTAP)
        .transpose(2, 0, 3, 1)
        .reshape(CIN, WCOLS)
        .astype(bf16)
    )
    cwb = np.concatenate([c_weight.T, bias[None, :]], axis=0)
    # host-baked SBUF image layout: leading zero guard element, XROWS rows
    # of stride W+1 with zero top/bottom halo rows and zero guard columns
    PWS = W + 1
    XCOLS = 1 + XROWS * PWS
    xbig = np.zeros((N_FULL, CIN, XCOLS), bf16)
    xbig[:, :, 1 + PWS : 1 + PWS + H * PWS].reshape(N_FULL, CIN, H, PWS)[
        :, :, :, :W
    ] = x
    in_maps = []
    for i in range(N_CORES):
        xs = np.ascontiguousarray(xbig[i * IMG : (i + 1) * IMG])
        cb = np.concatenate(
            [c[i * IMG : (i + 1) * IMG].T, np.ones((1, IMG), np.float32)], axis=0
        )
        cwbc = np.ascontiguousarray(
            np.concatenate([cwb, cb], axis=1).astype(bf16)
        )
        in_maps.append({"x": xs, "wt": wt, "cwbc": cwbc})
    return in_maps


def run(x, c, weight, c_weight, bias, trace=False):
    nc = get_nc()
    in_maps = prep_in_maps(x, c, weight, c_weight, bias)
    last_err = None
    for attempt in range(3):
        try:
            res = bass_utils.run_bass_kernel_spmd(
                nc, in_maps, core_ids=list(range(N_CORES)), trace=trace
            )
            break
        except Exception as e:  # noqa: BLE001
            # NRT_EXEC_UNIT_UNRECOVERABLE occasionally fires spuriously;
            # a reloaded execution recovers
            last_err = e
            time.sleep(2.0)
    else:
        raise last_err
    out = np.concatenate([res.results[i]["out"] for i in range(N_CORES)], axis=0)
    return out, res


def kernel(x, c, weight, c_weight, bias):
    out, _ = run(x, c, weight, c_weight, bias)
    return out


# revision 20
# speedup vs baseline: 1.0144x; 1.0002x over previous
"""ContextualConv2d Trainium2 kernel.

out = conv2d(x, weight, pad=1) + (c @ c_weight.T)[:, :, None, None] + bias[None, :, None, None]

Full shapes: x (32,128,64,64) f32, c (32,64), weight (256,128,3,3),
c_weight (256,64), bias (256,) -> out (32,256,64,64).

Strategy: data-parallel over batch across 8 NeuronCores (4 images each).
Per core the conv is an implicit GEMM: each image lives in SBUF with
stride-65 rows (a host-baked zero guard column after each 64-pixel row,
plus two zero rows for the H halo), so the +-1-column filter taps read
straight through zero guards and every tap is a uniform N=512 matmul
with inner-contiguous rhs. For each 128-wide C_out tile and each
512-column output block (8 image rows x 64 cols), 9 matmuls (one per
filter tap) accumulate into a PSUM bank.

Conv operands are bf16 (2.3e-3 rel err, well under the 2e-2 gate): bf16
moving streams at the full 1 elem/cycle PE rate with a ~219ns warm
back-to-back gap at N=512 (fp32r measured ~237ns - fp32-class
LDWEIGHTS is slower and only partially hidden), and input DMA bytes
halve. fp8 was measured numerically and rejected: e4m3 on both
operands gives 3.8e-2 rel err, over the gate, so DoubleRow's ~1.44x
is unreachable.

Head schedule (each DMA piece costs ~1.3us latency and the rings move
~100-110GB/s early, so pieces are ordered/sized by need time): x is
host-baked into its exact guard-padded SBUF layout so every piece is a
contiguous column slice; image 0 is interleaved across both rings in
8-row pieces (each ring feeds every other conv block), the
co-tile-major weights lead the scalar ring in three tap-group pieces,
and the merged context tensor rides behind them. 13 N=256 warmup
matmuls bridge the PE from preamble end (~7.7us) to the first conv
matmul (~10.4us) with <220ns handoff granularity so the HAM clock gate
un-throttles exactly at conv start. The ctx matmuls (bf16, with a
ones-row folding in the channel bias) sit between conv blocks 1 and 2
in the tensor queue; the bias is fused into the PSUM->SBUF epilogue on
ACT (co-tile 0) / DVE (co-tile 1). Images 1-3 are prefetched one
compute pass ahead on the scalar ring as single DMAs; output planes
are stored in 4 x 512KB pieces, except the last plane which goes in
8 x 256KB pieces with its final block's eviction split across DVE/ACT
and the final 256KB split across both rings, so the kernel tail only
carries ~128KB of store per ring.

Measured: ~144.0us HW exec (Core 0), vs ~159us for the fp32r baseline
and a ~124.5us PE-stream roofline (576 N=512 matmuls at 216ns); the
gap is ~7us fixed framework preamble/teardown plus ~3.5us of
DMA-latency-bound head.
"""

import sys
import time
import types

import numpy as np

import concourse.tile as tile
from concourse import bacc, bass_utils, mybir


def _ensure_axon_hooks_shim():
    """concourse imports antenv.axon_hooks when BASS_TRACE is set; the agent
    image's antenv lacks it. Provide a null shim so tracing degrades to a
    warning instead of an ImportError."""
    try:
        import antenv

        if not hasattr(antenv, "axon_hooks"):
            try:
                from antenv import axon_hooks  # noqa: F401
            except ImportError:
                mod = types.ModuleType("antenv.axon_hooks")
                _state = {"hook": None}
                mod.set_axon_ntff_profile_hook = lambda h: _state.__setitem__(
                    "hook", h
                )
                mod.get_axon_ntff_profile_hook = lambda: _state["hook"]
                sys.modules["antenv.axon_hooks"] = mod
                antenv.axon_hooks = mod
    except Exception:
        pass


_ensure_axon_hooks_shim()

N_CORES = 8
N_FULL = 32
IMG = N_FULL // N_CORES  # images per core
CIN = 128
COUT = 256
H = W = 64
HW = H * W
KDIM = 3
NTAP = KDIM * KDIM
CDIM = 64
XROWS = H + 2  # 2 zero rows for the H halo
CO_TILES = COUT // 128
ROWS_PER_BLK = 8
NBLK = H // ROWS_PER_BLK
BLK_N = ROWS_PER_BLK * W  # 512 = one fp32 PSUM bank
F32 = mybir.dt.float32
F32R = mybir.dt.float32r
BF16 = mybir.dt.bfloat16
WCOLS = CO_TILES * NTAP * 128  # co-tile-major weight layout

_cached_nc = None


def _build():
    nc = bacc.Bacc(
        "TRN2",
        target_bir_lowering=False,
        debug=False,
        enable_asserts=False,
        num_devices=N_CORES,
    )
    # x is host-baked into its exact SBUF layout per image: a leading zero
    # guard element, then XROWS rows of stride PWS=W+1 (top/bottom zero halo
    # rows included, zero guard column after each row) — so every image
    # loads with plain contiguous column-slice DMAs and no zero-fill pieces
    XCOLS = 1 + XROWS * (W + 1)
    x_d = nc.dram_tensor("x", (IMG, CIN, XCOLS), BF16, kind="ExternalInput").ap()
    wt_d = nc.dram_tensor("wt", (CIN, WCOLS), BF16, kind="ExternalInput").ap()
    # c/ones rows and c_weight/bias columns merged: [:, :COUT] is
    # [c_weight.T; bias], [:, COUT:] is [c.T; ones]
    cwbc_d = nc.dram_tensor(
        "cwbc", (CDIM + 1, COUT + IMG), BF16, kind="ExternalInput"
    ).ap()
    out_d = nc.dram_tensor("out", (IMG, COUT, H, W), F32, kind="ExternalOutput").ap()

    with tile.TileContext(nc) as tc:
        with (
            tc.tile_pool(name="consts", bufs=1) as consts,
            tc.tile_pool(name="xbuf", bufs=1) as xbuf,
            tc.tile_pool(name="obuf", bufs=2) as obuf,
            tc.tile_pool(name="ps", bufs=5, space="PSUM") as pspool,
            tc.tile_pool(name="cps", bufs=1, space="PSUM") as cpspool,
            tc.tile_pool(name="wps", bufs=1, space="PSUM") as wpspool,
        ):
            # PE warmup: the HAM clock gate needs ~3.4us of sustained matmul
            # activity to lift the 1.2GHz cold throttle, and any idle gap
            # between warmup and the first real matmul restarts the busy
            # window (costing ~2us of cold conv matmuls). 13 N=256 dummy
            # matmuls (~213ns each cold) bridge from the ~7.7us preamble end
            # to the ~10.5us conv start with fine granularity, so the
            # handoff gap stays under one matmul.
            warm_sb = consts.tile([CIN, BLK_N], BF16)
            nc.gpsimd.memset(warm_sb[:], 0.0)
            wps = wpspool.tile([128, BLK_N], F32)
            for _ in range(13):
                nc.tensor.matmul(
                    wps[:, 0 : BLK_N // 2],
                    lhsT=warm_sb[:, 0:128],
                    rhs=warm_sb[:, 0 : BLK_N // 2],
                    start=True,
                    stop=True,
                )

            # per-image input planes with stride-65 rows: position
            # 1 + u*PWS + c holds image pixel (u-1, c); column PWS-1 of each
            # row is a zero guard (baked into the host-padded x tensor), and
            # rows 0 / XROWS-1 plus the leading element are zeroed from z_d.
            # The +-1-column taps then read straight through the guards
            # (which contribute zero), so every tap is a uniform N=512
            # matmul with inner-contiguous rhs and a plain 2D PSUM out.
            PWS = W + 1

            def load_image(n, ring, cuts):
                """Emit the image-n load in len(cuts) column-slice pieces
                (cuts are exclusive xp-row upper bounds; the last must be
                XROWS). The host tensor already carries the halo/guards."""
                # one extra row of slack: tap AP slices extend past the last
                # guard before the [:, :, :W] crop trims them
                xp = xbuf.tile([CIN, 1 + (XROWS + 1) * PWS], BF16, tag=f"ximg{n}")
                a = 0
                for u in cuts:
                    b = 1 + u * PWS
                    ring.dma_start(out=xp[:, a:b], in_=x_d[n][:, a:b])
                    a = b
                return xp

            # Each DMA piece costs ~1.3us of fixed latency on its ring and
            # the rings deliver only ~100-110GB/s early, so image 0 is
            # interleaved across BOTH rings in 8-row pieces — each ring
            # supplies every other conv block and neither falls behind the
            # ~2us/block consumption rate. Weights lead the scalar ring in
            # three tap-group pieces (block-0 matmuls consume taps in
            # order); co-tile 1 rides later (needed only at ~27us).
            xp0 = xbuf.tile([CIN, 1 + (XROWS + 1) * PWS], BF16, tag="ximg0")
            xflats = {0: xp0}

            def x0_piece(ring, a, b):
                lo = 0 if a == 0 else 1 + a * PWS
                hi = 1 + b * PWS
                ring.dma_start(out=xp0[:, lo:hi], in_=x_d[0][:, lo:hi])

            w_sb = consts.tile([CIN, WCOLS], BF16)
            TG = NTAP * 128  # one co-tile's weight columns
            cwbc_sb = consts.tile([CDIM + 1, COUT + IMG], BF16)

            for c0, c1 in ((0, 384), (384, 768), (768, TG)):
                nc.scalar.dma_start(out=w_sb[:, c0:c1], in_=wt_d[:, c0:c1])
            x0_piece(nc.sync, 0, 10)
            x0_piece(nc.sync, 10, 18)
            x0_piece(nc.sync, 18, 26)
            nc.scalar.dma_start(out=cwbc_sb[:], in_=cwbc_d)
            x0_piece(nc.scalar, 26, 34)
            x0_piece(nc.sync, 34, 42)
            nc.scalar.dma_start(out=w_sb[:, TG:WCOLS], in_=wt_d[:, TG:WCOLS])
            x0_piece(nc.scalar, 42, 50)
            x0_piece(nc.sync, 50, 58)
            x0_piece(nc.scalar, 58, XROWS)

            # ctxb[t][co, n] = sum_d c_weight[co, d] * c[n, d] + bias[co];
            # emitted after conv block 0's matmuls so the conv start isn't
            # gated on the cwbc DMA (the first epilogue needs ctxb ~1us
            # after block 0 finishes — plenty)
            ctxb = []

            def emit_ctx():
                for t in range(CO_TILES):
                    cps = cpspool.tile([128, IMG], F32, tag=f"cps{t}")
                    nc.tensor.matmul(
                        cps[:],
                        lhsT=cwbc_sb[:, t * 128 : (t + 1) * 128],
                        rhs=cwbc_sb[:, COUT : COUT + IMG],
                        start=True,
                        stop=True,
                    )
                    csb = consts.tile([128, IMG], F32, tag=f"ctxb{t}")
                    nc.vector.tensor_copy(csb[:], cps[:])
                    ctxb.append(csb)

            for n in range(IMG):
                xf = xflats[n]
                for t in range(CO_TILES):
                    obig = obuf.tile([128, HW], F32)
                    pending = None
                    for b in range(NBLK):
                        ps = pspool.tile([128, BLK_N], F32)
                        r0 = b * ROWS_PER_BLK
                        for i in range(NTAP):
                            kh, kw = divmod(i, KDIM)
                            w0 = (t * NTAP + i) * 128
                            o = 1 + (r0 + kh) * PWS + (kw - 1)
                            rhs = xf[:, o : o + ROWS_PER_BLK * PWS].rearrange(
                                "p (r c) -> p r c", c=PWS
                            )[:, :, :W]
                            nc.tensor.matmul(
                                ps[:],
                                lhsT=w_sb[:, w0 : w0 + 128],
                                rhs=rhs,
                                start=(i == 0),
                                stop=(i == NTAP - 1),
                            )
                        if n == 0 and t == 0:
                            # the ctx matmuls sit between block 1 and block 2
                            # in the tensor queue, giving the cwbc DMA until
                            # ~14us to land; block 0's epilogue (which needs
                            # ctxb) is deferred until after they're emitted
                            if b == 0:
                                pending = ps
                                continue
                            if b == 1:
                                emit_ctx()
                                nc.scalar.activation(
                                    obig[:, 0:BLK_N],
                                    pending[:],
                                    mybir.ActivationFunctionType.Identity,
                                    bias=ctxb[0][:, 0:1],
                                    scale=1.0,
                                )
                        oslice = obig[:, b * BLK_N : (b + 1) * BLK_N]
                        last_blk = (
                            n == IMG - 1 and t == CO_TILES - 1 and b == NBLK - 1
                        )
                        if last_blk:
                            # split the final eviction across DVE and ACT so
                            # both halves of the tail store launch at once
                            hb = BLK_N // 2
                            nc.vector.tensor_scalar_add(
                                oslice[:, 0:hb], ps[:, 0:hb], ctxb[t][:, n : n + 1]
                            )
                            nc.scalar.activation(
                                oslice[:, hb:BLK_N],
                                ps[:, hb:BLK_N],
                                mybir.ActivationFunctionType.Identity,
                                bias=ctxb[t][:, n : n + 1],
                                scale=1.0,
                            )
                        elif t == 0:
                            nc.scalar.activation(
                                oslice,
                                ps[:],
                                mybir.ActivationFunctionType.Identity,
                                bias=ctxb[t][:, n : n + 1],
                                scale=1.0,
                            )
                        else:
                            nc.vector.tensor_scalar_add(
                                oslice, ps[:], ctxb[t][:, n : n + 1]
                            )
                    # split the 2MB plane store so the last piece doesn't sit
                    # whole on the kernel's critical tail; the very last
                    # plane goes in 8 pieces with the final 256KB split
                    # across both rings
                    oflat = out_d[n, t * 128 : (t + 1) * 128].rearrange(
                        "o h w -> o (h w)"
                    )
                    if n == IMG - 1 and t == CO_TILES - 1:
                        P8 = HW // 8
                        for q in range(7):
                            nc.sync.dma_start(
                                out=oflat[:, q * P8 : (q + 1) * P8],
                                in_=obig[:, q * P8 : (q + 1) * P8],
                            )
                        nc.sync.dma_start(
                            out=oflat[:, 7 * P8 : 7 * P8 + P8 // 2],
                            in_=obig[:, 7 * P8 : 7 * P8 + P8 // 2],
                        )
                        nc.scalar.dma_start(
                            out=oflat[:, 7 * P8 + P8 // 2 : HW],
                            in_=obig[:, 7 * P8 + P8 // 2 : HW],
                        )
                    else:
                        for q in range(4):
                            nc.sync.dma_start(
                                out=oflat[:, q * (HW // 4) : (q + 1) * (HW // 4)],
                                in_=obig[:, q * (HW // 4) : (q + 1) * (HW // 4)],
                            )
                    # prefetch the next image while this one's second
                    # C_out tile computes
                    if t == 0 and n + 1 < IMG:
                        xflats[n + 1] = load_image(n + 1, nc.scalar, (XROWS,))
    nc.compile()
    return nc


def get_nc():
    global _cached_nc
    if _cached_nc is None:
        _cached_nc = _build()
    return _cached_nc


def prep_in_maps(x, c, weight, c_weight, bias):
    import ml_dtypes

    bf16 = ml_dtypes.bfloat16
    x = np.ascontiguousarray(np.asarray(x, dtype=np.float32))
    c = np.asarray(c, dtype=np.float32)
    weight = np.asarray(weight, dtype=np.float32)
    c_weight = np.asarray(c_weight, dtype=np.float32)
    bias = np.asarray(bias, dtype=np.float32)

    # co-tile-major: wt[ci, t*NTAP*128 + i*128 + co] = weight[t*128+co, ci, i]
    wt = np.ascontiguousarray(
        weight.reshape(CO_TILES, 128, CIN, NTAP)
        .transpose(2, 0, 3, 1)
        .reshape(CIN, WCOLS)
        .astype(bf16)
    )
    cwb = np.concatenate([c_weight.T, bias[None, :]], axis=0)
    # host-baked SBUF image layout: leading zero guard element, XROWS rows
    # of stride W+1 with zero top/bottom halo rows and zero guard columns
    PWS = W + 1
    XCOLS = 1 + XROWS * PWS
    xbig = np.zeros((N_FULL, CIN, XCOLS), bf16)
    xbig[:, :, 1 + PWS : 1 + PWS + H * PWS].reshape(N_FULL, CIN, H, PWS)[
        :, :, :, :W
    ] = x
    in_maps = []
    for i in range(N_CORES):
        xs = np.ascontiguousarray(xbig[i * IMG : (i + 1) * IMG])
        cb = np.concatenate(
            [c[i * IMG : (i + 1) * IMG].T, np.ones((1, IMG), np.float32)], axis=0
        )
        cwbc = np.ascontiguousarray(
            np.concatenate([cwb, cb], axis=1).astype(bf16)
        )
        in_maps.append({"x": xs, "wt": wt, "cwbc": cwbc})
    return in_maps


def run(x, c, weight, c_weight, bias, trace=False):
    nc = get_nc()
    in_maps = prep_in_maps(x, c, weight, c_weight, bias)
    last_err = None
    for attempt in range(3):
        try:
            res = bass_utils.run_bass_kernel_spmd(
                nc, in_maps, core_ids=list(range(N_CORES)), trace=trace
            )
            break
        except Exception as e:  # noqa: BLE001
            # NRT_EXEC_UNIT_UNRECOVERABLE occasionally fires spuriously;
            # a reloaded execution recovers
            last_err = e
            time.sleep(2.0)
    else:
        raise last_err
    out = np.concatenate([res.results[i]["out"] for i in range(N_CORES)], axis=0)
    return out, res


def kernel(x, c, weight, c_weight, bias):
    out, _ = run(x, c, weight, c_weight, bias)
    return out


# revision 28
# speedup vs baseline: 1.0228x; 1.0082x over previous
"""ContextualConv2d Trainium2 kernel.

out = conv2d(x, weight, pad=1) + (c @ c_weight.T)[:, :, None, None] + bias[None, :, None, None]

Full shapes: x (32,128,64,64) f32, c (32,64), weight (256,128,3,3),
c_weight (256,64), bias (256,) -> out (32,256,64,64).

Strategy: data-parallel over batch across 8 NeuronCores (4 images each).
Per core the conv is an implicit GEMM: each image lives in SBUF with
stride-65 rows (a host-baked zero guard column after each 64-pixel row,
plus two zero rows for the H halo), so the +-1-column filter taps read
straight through zero guards and every tap is a uniform N=512 matmul
with inner-contiguous rhs. For each 128-wide C_out tile and each
512-column output block (8 image rows x 64 cols), 9 matmuls (one per
filter tap) accumulate into a PSUM bank.

Conv operands are bf16 (2.3e-3 rel err, well under the 2e-2 gate): bf16
moving streams at the full 1 elem/cycle PE rate with a ~219ns warm
back-to-back gap at N=512 (fp32r measured ~237ns - fp32-class
LDWEIGHTS is slower and only partially hidden), and input DMA bytes
halve. fp8 was measured numerically and rejected: e4m3 on both
operands gives 3.8e-2 rel err, over the gate, so DoubleRow's ~1.44x
is unreachable.

Head schedule (each DMA piece costs ~1.3us latency and the rings move
~100-110GB/s early, so pieces are ordered/sized by need time): x is
host-baked into its exact guard-padded SBUF layout so every piece is a
contiguous column slice; image 0 is interleaved across both rings in
8-row pieces (each ring feeds every other conv block), the
co-tile-major weights lead the scalar ring in three tap-group pieces,
and the merged context tensor rides behind them. 13 N=256 warmup
matmuls bridge the PE from preamble end (~7.7us) to the first conv
matmul (~10.4us) with <220ns handoff granularity so the HAM clock gate
un-throttles exactly at conv start. The ctx matmuls (bf16, with a
ones-row folding in the channel bias) sit between conv blocks 1 and 2
in the tensor queue; the bias is fused into the PSUM->SBUF epilogue on
ACT (co-tile 0) / DVE (co-tile 1). Images 1-3 are prefetched one
compute pass ahead on the scalar ring as single DMAs; output planes
are stored in 4 x 512KB pieces, except the last plane which goes in
8 x 256KB pieces with its final block's eviction split across DVE/ACT
and the final 256KB split across both rings, so the kernel tail only
carries ~128KB of store per ring.

Measured: ~144.0us HW exec (Core 0), vs ~159us for the fp32r baseline
and a ~124.5us PE-stream roofline (576 N=512 matmuls at 216ns); the
gap is ~7us fixed framework preamble/teardown plus ~3.5us of
DMA-latency-bound head.
"""

import sys
import time
import types

import numpy as np

import concourse.tile as tile
from concourse import bacc, bass_utils, mybir


def _ensure_axon_hooks_shim():
    """concourse imports antenv.axon_hooks when BASS_TRACE is set; the agent
    image's antenv lacks it. Provide a null shim so tracing degrades to a
    warning instead of an ImportError."""
    try:
        import antenv

        if not hasattr(antenv, "axon_hooks"):
            try:
                from antenv import axon_hooks  # noqa: F401
            except ImportError:
                mod = types.ModuleType("antenv.axon_hooks")
                _state = {"hook": None}
                mod.set_axon_ntff_profile_hook = lambda h: _state.__setitem__(
                    "hook", h
                )
                mod.get_axon_ntff_profile_hook = lambda: _state["hook"]
                sys.modules["antenv.axon_hooks"] = mod
                antenv.axon_hooks = mod
    except Exception:
        pass


_ensure_axon_hooks_shim()

N_CORES = 8
N_FULL = 32
IMG = N_FULL // N_CORES  # images per core
CIN = 128
COUT = 256
H = W = 64
HW = H * W
KDIM = 3
NTAP = KDIM * KDIM
CDIM = 64
XROWS = H + 2  # 2 zero rows for the H halo
CO_TILES = COUT // 128
ROWS_PER_BLK = 8
NBLK = H // ROWS_PER_BLK
BLK_N = ROWS_PER_BLK * W  # 512 = one fp32 PSUM bank
F32 = mybir.dt.float32
F32R = mybir.dt.float32r
BF16 = mybir.dt.bfloat16
WCOLS = CO_TILES * NTAP * 128  # co-tile-major weight layout

_cached_nc = None


def _build():
    nc = bacc.Bacc(
        "TRN2",
        target_bir_lowering=False,
        debug=False,
        enable_asserts=False,
        num_devices=N_CORES,
    )
    # x is host-baked into its exact SBUF layout per image: a leading zero
    # guard element, then XROWS rows of stride PWS=W+1 (top/bottom zero halo
    # rows included, zero guard column after each row) — so every image
    # loads with plain contiguous column-slice DMAs and no zero-fill pieces
    XCOLS = 1 + XROWS * (W + 1)
    x_d = nc.dram_tensor("x", (IMG, CIN, XCOLS), BF16, kind="ExternalInput").ap()
    wt_d = nc.dram_tensor("wt", (CIN, WCOLS), BF16, kind="ExternalInput").ap()
    # c/ones rows and c_weight/bias columns merged: [:, :COUT] is
    # [c_weight.T; bias], [:, COUT:] is [c.T; ones]
    cwbc_d = nc.dram_tensor(
        "cwbc", (CDIM + 1, COUT + IMG), BF16, kind="ExternalInput"
    ).ap()
    out_d = nc.dram_tensor("out", (IMG, COUT, H, W), F32, kind="ExternalOutput").ap()

    with tile.TileContext(nc) as tc:
        with (
            tc.tile_pool(name="consts", bufs=1) as consts,
            tc.tile_pool(name="xbuf", bufs=1) as xbuf,
            tc.tile_pool(name="obuf", bufs=2) as obuf,
            tc.tile_pool(name="ps", bufs=5, space="PSUM") as pspool,
            tc.tile_pool(name="cps", bufs=1, space="PSUM") as cpspool,
            tc.tile_pool(name="wps", bufs=1, space="PSUM") as wpspool,
        ):
            # PE warmup: the HAM clock gate needs ~3.4us of sustained matmul
            # activity to lift the 1.2GHz cold throttle, and any idle gap
            # between warmup and the first real matmul restarts the busy
            # window (costing ~2us of cold conv matmuls). 13 N=256 dummy
            # matmuls (~213ns each cold) bridge from the ~7.7us preamble end
            # to the ~10.5us conv start with fine granularity, so the
            # handoff gap stays under one matmul.
            warm_sb = consts.tile([CIN, BLK_N], BF16)
            nc.gpsimd.memset(warm_sb[:], 0.0)
            wps = wpspool.tile([128, BLK_N], F32)
            for _ in range(13):
                nc.tensor.matmul(
                    wps[:, 0 : BLK_N // 2],
                    lhsT=warm_sb[:, 0:128],
                    rhs=warm_sb[:, 0 : BLK_N // 2],
                    start=True,
                    stop=True,
                )

            # per-image input planes with stride-65 rows: position
            # 1 + u*PWS + c holds image pixel (u-1, c); column PWS-1 of each
            # row is a zero guard (baked into the host-padded x tensor), and
            # rows 0 / XROWS-1 plus the leading element are zeroed from z_d.
            # The +-1-column taps then read straight through the guards
            # (which contribute zero), so every tap is a uniform N=512
            # matmul with inner-contiguous rhs and a plain 2D PSUM out.
            PWS = W + 1

            def load_image(n, ring, cuts):
                """Emit the image-n load in len(cuts) column-slice pieces
                (cuts are exclusive xp-row upper bounds; the last must be
                XROWS). The host tensor already carries the halo/guards."""
                # one extra row of slack: tap AP slices extend past the last
                # guard before the [:, :, :W] crop trims them
                xp = xbuf.tile([CIN, 1 + (XROWS + 1) * PWS], BF16, tag=f"ximg{n}")
                a = 0
                for u in cuts:
                    b = 1 + u * PWS
                    ring.dma_start(out=xp[:, a:b], in_=x_d[n][:, a:b])
                    a = b
                return xp

            # Each DMA piece costs ~1.3us of fixed latency on its ring and
            # the rings deliver only ~100-110GB/s early, so image 0 is
            # interleaved across BOTH rings in 8-row pieces — each ring
            # supplies every other conv block and neither falls behind the
            # ~2us/block consumption rate. Weights lead the scalar ring in
            # three tap-group pieces (block-0 matmuls consume taps in
            # order); co-tile 1 rides later (needed only at ~27us).
            xp0 = xbuf.tile([CIN, 1 + (XROWS + 1) * PWS], BF16, tag="ximg0")
            xflats = {0: xp0}

            def x0_piece(ring, a, b):
                lo = 0 if a == 0 else 1 + a * PWS
                hi = 1 + b * PWS
                ring.dma_start(out=xp0[:, lo:hi], in_=x_d[0][:, lo:hi])

            w_sb = consts.tile([CIN, WCOLS], BF16)
            TG = NTAP * 128  # one co-tile's weight columns
            cwbc_sb = consts.tile([CDIM + 1, COUT + IMG], BF16)

            for c0, c1 in ((0, 384), (384, 768), (768, TG)):
                nc.scalar.dma_start(out=w_sb[:, c0:c1], in_=wt_d[:, c0:c1])
            x0_piece(nc.sync, 0, 10)
            x0_piece(nc.sync, 10, 18)
            x0_piece(nc.sync, 18, 26)
            nc.scalar.dma_start(out=cwbc_sb[:], in_=cwbc_d)
            x0_piece(nc.scalar, 26, 34)
            x0_piece(nc.sync, 34, 42)
            nc.scalar.dma_start(out=w_sb[:, TG:WCOLS], in_=wt_d[:, TG:WCOLS])
            x0_piece(nc.scalar, 42, 50)
            x0_piece(nc.sync, 50, 58)
            x0_piece(nc.scalar, 58, XROWS)

            # ctxb[t][co, n] = sum_d c_weight[co, d] * c[n, d] + bias[co];
            # emitted after conv block 0's matmuls so the conv start isn't
            # gated on the cwbc DMA (the first epilogue needs ctxb ~1us
            # after block 0 finishes — plenty)
            ctxb = []

            def emit_ctx():
                for t in range(CO_TILES):
                    cps = cpspool.tile([128, IMG], F32, tag=f"cps{t}")
                    nc.tensor.matmul(
                        cps[:],
                        lhsT=cwbc_sb[:, t * 128 : (t + 1) * 128],
                        rhs=cwbc_sb[:, COUT : COUT + IMG],
                        start=True,
                        stop=True,
                    )
                    csb = consts.tile([128, IMG], F32, tag=f"ctxb{t}")
                    nc.vector.tensor_copy(csb[:], cps[:])
                    ctxb.append(csb)

            for n in range(IMG):
                xf = xflats[n]
                for t in range(CO_TILES):
                    obig = obuf.tile([128, HW], F32)
                    pending = None
                    last_plane = n == IMG - 1 and t == CO_TILES - 1
                    # on the last plane the final 8-row block runs as two
                    # 4-row halves (same stream time) so the tail's final
                    # evict+store chain carries only 64KB per ring
                    specs = [(b * ROWS_PER_BLK, ROWS_PER_BLK) for b in range(NBLK)]
                    if last_plane:
                        r7 = (NBLK - 1) * ROWS_PER_BLK
                        hr = ROWS_PER_BLK // 2
                        specs = specs[:-1] + [(r7, hr), (r7 + hr, hr)]
                    for k, (r0, nrows) in enumerate(specs):
                        bn = nrows * W
                        ps = pspool.tile([128, BLK_N], F32)
                        for i in range(NTAP):
                            kh, kw = divmod(i, KDIM)
                            w0 = (t * NTAP + i) * 128
                            o = 1 + (r0 + kh) * PWS + (kw - 1)
                            rhs = xf[:, o : o + nrows * PWS].rearrange(
                                "p (r c) -> p r c", c=PWS
                            )[:, :, :W]
                            nc.tensor.matmul(
                                ps[:, 0:bn],
                                lhsT=w_sb[:, w0 : w0 + 128],
                                rhs=rhs,
                                start=(i == 0),
                                stop=(i == NTAP - 1),
                            )
                        if n == 0 and t == 0:
                            # the ctx matmuls sit between block 1 and block 2
                            # in the tensor queue, giving the cwbc DMA until
                            # ~14us to land; block 0's epilogue (which needs
                            # ctxb) is deferred until after they're emitted
                            if k == 0:
                                pending = ps
                                continue
                            if k == 1:
                                emit_ctx()
                                nc.scalar.activation(
                                    obig[:, 0:BLK_N],
                                    pending[:],
                                    mybir.ActivationFunctionType.Identity,
                                    bias=ctxb[0][:, 0:1],
                                    scale=1.0,
                                )
                        oslice = obig[:, r0 * W : r0 * W + bn]
                        # epilogue engine: ACT for co-tile 0, DVE for co-tile
                        # 1 — except the two tail halves, which go DVE then
                        # ACT so the two final stores launch from different
                        # rings as soon as each half lands
                        if nrows != ROWS_PER_BLK:
                            use_act = k == len(specs) - 1
                        else:
                            use_act = t == 0
                        if use_act:
                            nc.scalar.activation(
                                oslice,
                                ps[:, 0:bn],
                                mybir.ActivationFunctionType.Identity,
                                bias=ctxb[t][:, n : n + 1],
                                scale=1.0,
                            )
                        else:
                            nc.vector.tensor_scalar_add(
                                oslice, ps[:, 0:bn], ctxb[t][:, n : n + 1]
                            )
                    # split the 2MB plane store so the last piece doesn't sit
                    # whole on the kernel's critical tail; the very last
                    # plane goes in 8 pieces with the final 256KB split
                    # across both rings
                    oflat = out_d[n, t * 128 : (t + 1) * 128].rearrange(
                        "o h w -> o (h w)"
                    )
                    if n == IMG - 1 and t == CO_TILES - 1:
                        P8 = HW // 8
                        for q in range(7):
                            nc.sync.dma_start(
                                out=oflat[:, q * P8 : (q + 1) * P8],
                                in_=obig[:, q * P8 : (q + 1) * P8],
                            )
                        nc.sync.dma_start(
                            out=oflat[:, 7 * P8 : 7 * P8 + P8 // 2],
                            in_=obig[:, 7 * P8 : 7 * P8 + P8 // 2],
                        )
                        nc.scalar.dma_start(
                            out=oflat[:, 7 * P8 + P8 // 2 : HW],
                            in_=obig[:, 7 * P8 + P8 // 2 : HW],
                        )
                    else:
                        for q in range(4):
                            nc.sync.dma_start(
                                out=oflat[:, q * (HW // 4) : (q + 1) * (HW // 4)],
                                in_=obig[:, q * (HW // 4) : (q + 1) * (HW // 4)],
                            )
                    # prefetch the next image while this one's second
                    # C_out tile computes
                    if t == 0 and n + 1 < IMG:
                        xflats[n + 1] = load_image(n + 1, nc.scalar, (XROWS,))
    nc.compile()
    return nc


def get_nc():
    global _cached_nc
    if _cached_nc is None:
        _cached_nc = _build()
    return _cached_nc


def prep_in_maps(x, c, weight, c_weight, bias):
    import ml_dtypes

    bf16 = ml_dtypes.bfloat16
    x = np.ascontiguousarray(np.asarray(x, dtype=np.float32))
    c = np.asarray(c, dtype=np.float32)
    weight = np.asarray(weight, dtype=np.float32)
    c_weight = np.asarray(c_weight, dtype=np.float32)
    bias = np.asarray(bias, dtype=np.float32)

    # co-tile-major: wt[ci, t*NTAP*128 + i*128 + co] = weight[t*128+co, ci, i]
    wt = np.ascontiguousarray(
        weight.reshape(CO_TILES, 128, CIN, NTAP)
        .transpose(2, 0, 3, 1)
        .reshape(CIN, WCOLS)
        .astype(bf16)
    )
    cwb = np.concatenate([c_weight.T, bias[None, :]], axis=0)
    # host-baked SBUF image layout: leading zero guard element, XROWS rows
    # of stride W+1 with zero top/bottom halo rows and zero guard columns
    PWS = W + 1
    XCOLS = 1 + XROWS * PWS
    xbig = np.zeros((N_FULL, CIN, XCOLS), bf16)
    xbig[:, :, 1 + PWS : 1 + PWS + H * PWS].reshape(N_FULL, CIN, H, PWS)[
        :, :, :, :W
    ] = x
    in_maps = []
    for i in range(N_CORES):
        xs = np.ascontiguousarray(xbig[i * IMG : (i + 1) * IMG])
        cb = np.concatenate(
            [c[i * IMG : (i + 1) * IMG].T, np.ones((1, IMG), np.float32)], axis=0
        )
        cwbc = np.ascontiguousarray(
            np.concatenate([cwb, cb], axis=1).astype(bf16)
        )
        in_maps.append({"x": xs, "wt": wt, "cwbc": cwbc})
    return in_maps


def run(x, c, weight, c_weight, bias, trace=False):
    nc = get_nc()
    in_maps = prep_in_maps(x, c, weight, c_weight, bias)
    last_err = None
    for attempt in range(3):
        try:
            res = bass_utils.run_bass_kernel_spmd(
                nc, in_maps, core_ids=list(range(N_CORES)), trace=trace
            )
            break
        except Exception as e:  # noqa: BLE001
            # NRT_EXEC_UNIT_UNRECOVERABLE occasionally fires spuriously;
            # a reloaded execution recovers
            last_err = e
            time.sleep(2.0)
    else:
        raise last_err
    out = np.concatenate([res.results[i]["out"] for i in range(N_CORES)], axis=0)
    return out, res


def kernel(x, c, weight, c_weight, bias):
    out, _ = run(x, c, weight, c_weight, bias)
    return out


# revision 31
# speedup vs baseline: 1.0231x; 1.0004x over previous
"""ContextualConv2d Trainium2 kernel.

out = conv2d(x, weight, pad=1) + (c @ c_weight.T)[:, :, None, None] + bias[None, :, None, None]

Full shapes: x (32,128,64,64) f32, c (32,64), weight (256,128,3,3),
c_weight (256,64), bias (256,) -> out (32,256,64,64).

Strategy: data-parallel over batch across 8 NeuronCores (4 images each).
Per core the conv is an implicit GEMM: each image lives in SBUF with
stride-65 rows (a host-baked zero guard column after each 64-pixel row,
plus two zero rows for the H halo), so the +-1-column filter taps read
straight through zero guards and every tap is a uniform N=512 matmul
with inner-contiguous rhs. For each 128-wide C_out tile and each
512-column output block (8 image rows x 64 cols), 9 matmuls (one per
filter tap) accumulate into a PSUM bank.

Conv operands are bf16 (2.3e-3 rel err, well under the 2e-2 gate): bf16
moving streams at the full 1 elem/cycle PE rate with a ~219ns warm
back-to-back gap at N=512 (fp32r measured ~237ns - fp32-class
LDWEIGHTS is slower and only partially hidden), and input DMA bytes
halve. fp8 was measured numerically and rejected: e4m3 on both
operands gives 3.8e-2 rel err, over the gate, so DoubleRow's ~1.44x
is unreachable.

Head schedule (each DMA piece costs ~1.3us latency and the rings move
~100-110GB/s early, so pieces are ordered/sized by need time): x is
host-baked into its exact guard-padded SBUF layout so every piece is a
contiguous column slice; image 0 is interleaved across both rings in
8-row pieces (each ring feeds every other conv block), the
co-tile-major weights lead the scalar ring in three tap-group pieces,
and the merged context tensor rides behind them. 13 N=256 warmup
matmuls bridge the PE from preamble end (~7.7us) to the first conv
matmul (~10.4us) with <220ns handoff granularity so the HAM clock gate
un-throttles exactly at conv start. The ctx matmuls (bf16, with a
ones-row folding in the channel bias) sit between conv blocks 1 and 2
in the tensor queue; the bias is fused into the PSUM->SBUF epilogue on
ACT (co-tile 0) / DVE (co-tile 1). Images 1-3 are prefetched one
compute pass ahead on the scalar ring as single DMAs; output planes
are stored in 4 x 512KB pieces, except the last plane which goes in
8 x 256KB pieces, its final 8-row block computed as two 4-row halves
(DVE->sync ring, then ACT->scalar ring) so the kernel tail only
carries ~64KB of store per ring after the last matmul.

Measured: ~143.4us HW exec (Core 0), vs ~159us for the fp32r baseline
and a ~124.5us PE-stream roofline (576 N=512 matmuls at 216ns); the
gap is ~7us fixed framework preamble/teardown plus ~3.5us of
DMA-latency-bound head.
"""

import sys
import time
import types

import numpy as np

import concourse.tile as tile
from concourse import bacc, bass_utils, mybir


def _ensure_axon_hooks_shim():
    """concourse imports antenv.axon_hooks when BASS_TRACE is set; the agent
    image's antenv lacks it. Provide a null shim so tracing degrades to a
    warning instead of an ImportError."""
    try:
        import antenv

        if not hasattr(antenv, "axon_hooks"):
            try:
                from antenv import axon_hooks  # noqa: F401
            except ImportError:
                mod = types.ModuleType("antenv.axon_hooks")
                _state = {"hook": None}
                mod.set_axon_ntff_profile_hook = lambda h: _state.__setitem__(
                    "hook", h
                )
                mod.get_axon_ntff_profile_hook = lambda: _state["hook"]
                sys.modules["antenv.axon_hooks"] = mod
                antenv.axon_hooks = mod
    except Exception:
        pass


_ensure_axon_hooks_shim()

N_CORES = 8
N_FULL = 32
IMG = N_FULL // N_CORES  # images per core
CIN = 128
COUT = 256
H = W = 64
HW = H * W
KDIM = 3
NTAP = KDIM * KDIM
CDIM = 64
XROWS = H + 2  # 2 zero rows for the H halo
CO_TILES = COUT // 128
ROWS_PER_BLK = 8
NBLK = H // ROWS_PER_BLK
BLK_N = ROWS_PER_BLK * W  # 512 = one fp32 PSUM bank
F32 = mybir.dt.float32
F32R = mybir.dt.float32r
BF16 = mybir.dt.bfloat16
WCOLS = CO_TILES * NTAP * 128  # co-tile-major weight layout

_cached_nc = None


def _build():
    nc = bacc.Bacc(
        "TRN2",
        target_bir_lowering=False,
        debug=False,
        enable_asserts=False,
        num_devices=N_CORES,
    )
    # x is host-baked into its exact SBUF layout per image: a leading zero
    # guard element, then XROWS rows of stride PWS=W+1 (top/bottom zero halo
    # rows included, zero guard column after each row) — so every image
    # loads with plain contiguous column-slice DMAs and no zero-fill pieces
    XCOLS = 1 + XROWS * (W + 1)
    x_d = nc.dram_tensor("x", (IMG, CIN, XCOLS), BF16, kind="ExternalInput").ap()
    wt_d = nc.dram_tensor("wt", (CIN, WCOLS), BF16, kind="ExternalInput").ap()
    # c/ones rows and c_weight/bias columns merged: [:, :COUT] is
    # [c_weight.T; bias], [:, COUT:] is [c.T; ones]
    cwbc_d = nc.dram_tensor(
        "cwbc", (CDIM + 1, COUT + IMG), BF16, kind="ExternalInput"
    ).ap()
    out_d = nc.dram_tensor("out", (IMG, COUT, H, W), F32, kind="ExternalOutput").ap()

    with tile.TileContext(nc) as tc:
        with (
            tc.tile_pool(name="consts", bufs=1) as consts,
            tc.tile_pool(name="xbuf", bufs=1) as xbuf,
            tc.tile_pool(name="obuf", bufs=2) as obuf,
            tc.tile_pool(name="ps", bufs=5, space="PSUM") as pspool,
            tc.tile_pool(name="cps", bufs=1, space="PSUM") as cpspool,
            tc.tile_pool(name="wps", bufs=1, space="PSUM") as wpspool,
        ):
            # PE warmup: the HAM clock gate needs ~3.4us of sustained matmul
            # activity to lift the 1.2GHz cold throttle, and any idle gap
            # between warmup and the first real matmul restarts the busy
            # window (costing ~2us of cold conv matmuls). 13 N=256 dummy
            # matmuls (~213ns each cold) bridge from the ~7.7us preamble end
            # to the ~10.5us conv start with fine granularity, so the
            # handoff gap stays under one matmul.
            warm_sb = consts.tile([CIN, BLK_N], BF16)
            nc.gpsimd.memset(warm_sb[:], 0.0)
            wps = wpspool.tile([128, BLK_N], F32)
            for _ in range(13):
                nc.tensor.matmul(
                    wps[:, 0 : BLK_N // 2],
                    lhsT=warm_sb[:, 0:128],
                    rhs=warm_sb[:, 0 : BLK_N // 2],
                    start=True,
                    stop=True,
                )

            # per-image input planes with stride-65 rows: position
            # 1 + u*PWS + c holds image pixel (u-1, c); column PWS-1 of each
            # row is a zero guard (baked into the host-padded x tensor), and
            # rows 0 / XROWS-1 plus the leading element are zeroed from z_d.
            # The +-1-column taps then read straight through the guards
            # (which contribute zero), so every tap is a uniform N=512
            # matmul with inner-contiguous rhs and a plain 2D PSUM out.
            PWS = W + 1

            def load_image(n, ring, cuts):
                """Emit the image-n load in len(cuts) column-slice pieces
                (cuts are exclusive xp-row upper bounds; the last must be
                XROWS). The host tensor already carries the halo/guards."""
                # one extra row of slack: tap AP slices extend past the last
                # guard before the [:, :, :W] crop trims them
                xp = xbuf.tile([CIN, 1 + (XROWS + 1) * PWS], BF16, tag=f"ximg{n}")
                a = 0
                for u in cuts:
                    b = 1 + u * PWS
                    ring.dma_start(out=xp[:, a:b], in_=x_d[n][:, a:b])
                    a = b
                return xp

            # Each DMA piece costs ~1.3us of fixed latency on its ring and
            # the rings deliver only ~100-110GB/s early, so image 0 is
            # interleaved across BOTH rings in 8-row pieces — each ring
            # supplies every other conv block and neither falls behind the
            # ~2us/block consumption rate. Weights lead the scalar ring in
            # three tap-group pieces (block-0 matmuls consume taps in
            # order); co-tile 1 rides later (needed only at ~27us).
            xp0 = xbuf.tile([CIN, 1 + (XROWS + 1) * PWS], BF16, tag="ximg0")
            xflats = {0: xp0}

            def x0_piece(ring, a, b):
                lo = 0 if a == 0 else 1 + a * PWS
                hi = 1 + b * PWS
                ring.dma_start(out=xp0[:, lo:hi], in_=x_d[0][:, lo:hi])

            w_sb = consts.tile([CIN, WCOLS], BF16)
            TG = NTAP * 128  # one co-tile's weight columns
            cwbc_sb = consts.tile([CDIM + 1, COUT + IMG], BF16)

            # taps 3-5 ride the gpsimd queue (idle during the head, and the
            # only other DMA-capable engine) so the scalar ring — which
            # alone can't feed block 0's 219ns/tap consumption rate — only
            # has to deliver taps 0-2 before block 0 starts and taps 6-8
            # one piece later
            for ring, (c0, c1) in zip(
                (nc.scalar, nc.gpsimd, nc.scalar),
                ((0, 384), (384, 768), (768, TG)),
            ):
                ring.dma_start(out=w_sb[:, c0:c1], in_=wt_d[:, c0:c1])
            x0_piece(nc.sync, 0, 10)
            x0_piece(nc.sync, 10, 18)
            x0_piece(nc.sync, 18, 26)
            nc.scalar.dma_start(out=cwbc_sb[:], in_=cwbc_d)
            x0_piece(nc.scalar, 26, 34)
            x0_piece(nc.sync, 34, 42)
            nc.scalar.dma_start(out=w_sb[:, TG:WCOLS], in_=wt_d[:, TG:WCOLS])
            x0_piece(nc.scalar, 42, 50)
            x0_piece(nc.sync, 50, 58)
            x0_piece(nc.scalar, 58, XROWS)

            # ctxb[t][co, n] = sum_d c_weight[co, d] * c[n, d] + bias[co];
            # emitted after conv block 0's matmuls so the conv start isn't
            # gated on the cwbc DMA (the first epilogue needs ctxb ~1us
            # after block 0 finishes — plenty)
            ctxb = []

            def emit_ctx():
                for t in range(CO_TILES):
                    cps = cpspool.tile([128, IMG], F32, tag=f"cps{t}")
                    nc.tensor.matmul(
                        cps[:],
                        lhsT=cwbc_sb[:, t * 128 : (t + 1) * 128],
                        rhs=cwbc_sb[:, COUT : COUT + IMG],
                        start=True,
                        stop=True,
                    )
                    csb = consts.tile([128, IMG], F32, tag=f"ctxb{t}")
                    nc.vector.tensor_copy(csb[:], cps[:])
                    ctxb.append(csb)

            for n in range(IMG):
                xf = xflats[n]
                for t in range(CO_TILES):
                    obig = obuf.tile([128, HW], F32)
                    pending = None
                    last_plane = n == IMG - 1 and t == CO_TILES - 1
                    # on the last plane the final 8-row block runs as two
                    # 4-row halves (same stream time) so the tail's final
                    # evict+store chain carries only 64KB per ring
                    specs = [(b * ROWS_PER_BLK, ROWS_PER_BLK) for b in range(NBLK)]
                    if last_plane:
                        r7 = (NBLK - 1) * ROWS_PER_BLK
                        hr = ROWS_PER_BLK // 2
                        specs = specs[:-1] + [(r7, hr), (r7 + hr, hr)]
                    for k, (r0, nrows) in enumerate(specs):
                        bn = nrows * W
                        ps = pspool.tile([128, BLK_N], F32)
                        for i in range(NTAP):
                            kh, kw = divmod(i, KDIM)
                            w0 = (t * NTAP + i) * 128
                            o = 1 + (r0 + kh) * PWS + (kw - 1)
                            rhs = xf[:, o : o + nrows * PWS].rearrange(
                                "p (r c) -> p r c", c=PWS
                            )[:, :, :W]
                            nc.tensor.matmul(
                                ps[:, 0:bn],
                                lhsT=w_sb[:, w0 : w0 + 128],
                                rhs=rhs,
                                start=(i == 0),
                                stop=(i == NTAP - 1),
                            )
                        if n == 0 and t == 0:
                            # the ctx matmuls sit between block 1 and block 2
                            # in the tensor queue, giving the cwbc DMA until
                            # ~14us to land; block 0's epilogue (which needs
                            # ctxb) is deferred until after they're emitted
                            if k == 0:
                                pending = ps
                                continue
                            if k == 1:
                                emit_ctx()
                                nc.scalar.activation(
                                    obig[:, 0:BLK_N],
                                    pending[:],
                                    mybir.ActivationFunctionType.Identity,
                                    bias=ctxb[0][:, 0:1],
                                    scale=1.0,
                                )
                        oslice = obig[:, r0 * W : r0 * W + bn]
                        # epilogue engine: ACT for co-tile 0, DVE for co-tile
                        # 1 — except the two tail halves, which go DVE then
                        # ACT so the two final stores launch from different
                        # rings as soon as each half lands
                        if nrows != ROWS_PER_BLK:
                            use_act = k == len(specs) - 1
                        else:
                            use_act = t == 0
                        if use_act:
                            nc.scalar.activation(
                                oslice,
                                ps[:, 0:bn],
                                mybir.ActivationFunctionType.Identity,
                                bias=ctxb[t][:, n : n + 1],
                                scale=1.0,
                            )
                        else:
                            nc.vector.tensor_scalar_add(
                                oslice, ps[:, 0:bn], ctxb[t][:, n : n + 1]
                            )
                    # split the 2MB plane store so the last piece doesn't sit
                    # whole on the kernel's critical tail; the very last
                    # plane goes in 8 pieces with the final 256KB split
                    # across both rings
                    oflat = out_d[n, t * 128 : (t + 1) * 128].rearrange(
                        "o h w -> o (h w)"
                    )
                    if n == IMG - 1 and t == CO_TILES - 1:
                        P8 = HW // 8
                        for q in range(7):
                            nc.sync.dma_start(
                                out=oflat[:, q * P8 : (q + 1) * P8],
                                in_=obig[:, q * P8 : (q + 1) * P8],
                            )
                        nc.sync.dma_start(
                            out=oflat[:, 7 * P8 : 7 * P8 + P8 // 2],
                            in_=obig[:, 7 * P8 : 7 * P8 + P8 // 2],
                        )
                        nc.scalar.dma_start(
                            out=oflat[:, 7 * P8 + P8 // 2 : HW],
                            in_=obig[:, 7 * P8 + P8 // 2 : HW],
                        )
                    else:
                        for q in range(4):
                            nc.sync.dma_start(
                                out=oflat[:, q * (HW // 4) : (q + 1) * (HW // 4)],
                                in_=obig[:, q * (HW // 4) : (q + 1) * (HW // 4)],
                            )
                    # prefetch the next image while this one's second
                    # C_out tile computes
                    if t == 0 and n + 1 < IMG:
                        xflats[n + 1] = load_image(n + 1, nc.scalar, (XROWS,))
    nc.compile()
    return nc


def get_nc():
    global _cached_nc
    if _cached_nc is None:
        _cached_nc = _build()
    return _cached_nc


def prep_in_maps(x, c, weight, c_weight, bias):
    import ml_dtypes

    bf16 = ml_dtypes.bfloat16
    x = np.ascontiguousarray(np.asarray(x, dtype=np.float32))
    c = np.asarray(c, dtype=np.float32)
    weight = np.asarray(weight, dtype=np.float32)
    c_weight = np.asarray(c_weight, dtype=np.float32)
    bias = np.asarray(bias, dtype=np.float32)

    # co-tile-major: wt[ci, t*NTAP*128 + i*128 + co] = weight[t*128+co, ci, i]
    wt = np.ascontiguousarray(
        weight.reshape(CO_TILES, 128, CIN, NTAP)
        .transpose(2, 0, 3, 1)
        .reshape(CIN, WCOLS)
        .astype(bf16)
    )
    cwb = np.concatenate([c_weight.T, bias[None, :]], axis=0)
    # host-baked SBUF image layout: leading zero guard element, XROWS rows
    # of stride W+1 with zero top/bottom halo rows and zero guard columns
    PWS = W + 1
    XCOLS = 1 + XROWS * PWS
    xbig = np.zeros((N_FULL, CIN, XCOLS), bf16)
    xbig[:, :, 1 + PWS : 1 + PWS + H * PWS].reshape(N_FULL, CIN, H, PWS)[
        :, :, :, :W
    ] = x
    in_maps = []
    for i in range(N_CORES):
        xs = np.ascontiguousarray(xbig[i * IMG : (i + 1) * IMG])
        cb = np.concatenate(
            [c[i * IMG : (i + 1) * IMG].T, np.ones((1, IMG), np.float32)], axis=0
        )
        cwbc = np.ascontiguousarray(
            np.concatenate([cwb, cb], axis=1).astype(bf16)
        )
        in_maps.append({"x": xs, "wt": wt, "cwbc": cwbc})
    return in_maps


def run(x, c, weight, c_weight, bias, trace=False):
    nc = get_nc()
    in_maps = prep_in_maps(x, c, weight, c_weight, bias)
    last_err = None
    for attempt in range(3):
        try:
            res = bass_utils.run_bass_kernel_spmd(
                nc, in_maps, core_ids=list(range(N_CORES)), trace=trace
            )
            break
        except Exception as e:  # noqa: BLE001
            # NRT_EXEC_UNIT_UNRECOVERABLE occasionally fires spuriously;
            # a reloaded execution recovers
            last_err = e
            time.sleep(2.0)
    else:
        raise last_err
    out = np.concatenate([res.results[i]["out"] for i in range(N_CORES)], axis=0)
    return out, res


def kernel(x, c, weight, c_weight, bias):
    out, _ = run(x, c, weight, c_weight, bias)
    return out
